# revision 28
# baseline (speedup 1.0000x reference)
"""Trainium2 Bass kernel for a latent ConvCNP (gaussian encoder -> CNN ->
latent samples -> gaussian interpolator), data-parallel over batch on 8
NeuronCores.

v5: sorted + pair-merged windowed encoder (see v4 notes) with packed input
DMAs (3 constant packs + 2 per-batch packs instead of 21 transfers),
per-channel table pipelining, a single merged h_grid psum + add, a
manually placed natural_log_exp act-table load (3 table loads total), and
per-batch softplus + output DMAs so batch 0's results leave the core while
batch 1 is still in flight.

Contract: kernel(**inputs) takes the full unsharded inputs (numpy) and
returns the full (NS, nb, ntar, 2C) output.
"""

import sys

sys.path.insert(0, "/opt/trn_rl_repo")

import math

import ml_dtypes
import numpy as np

import concourse.bacc as bacc
import concourse.mybir as mybir
import concourse.tile as tile
from concourse import bass_utils
from concourse.ap import AP
from concourse.tile_rust import add_dep_helper

F32 = mybir.dt.float32
F32R = mybir.dt.float32r
BF16 = mybir.dt.bfloat16
AF = mybir.ActivationFunctionType
ALU = mybir.AluOpType

# problem constants (fixed by the reference problem)
EPS = 1e-6
C = 3
NBASIS = 5
NS = 4
RIN = 16
ROUT = 32
KW = 5
NB = 16          # full batch
NPTS = 2048
NTAR = 256
NCORES = 8
NBL = NB // NCORES   # batches per core
NPM = NPTS // 2      # pair-merged points
NCH = NPM // 128     # 8 chunks per (b, c)
KAPPA = math.sqrt(math.pi) / 2.0  # exp(-x^2) = KAPPA * Derivative_Erf(x)
KREACH = 4.0                      # window reach in units of 1/alpha
NF = C * NBASIS
NTT = NTAR // 128
W24 = NS * 2 * C

_CACHE = {}


def _build(m, q, aoff, wf, mext, moff):
    """Per-core Bass program. m = grid cols; chunk ch's window occupies psum
    cols [aoff + q*ch, +wf); grid col j lives at psum col j + moff (the gx
    input content is shifted to match)."""
    njt = (m + 127) // 128
    mts = [128] * (m // 128) + ([m % 128] if m % 128 else [])
    mp = m + 4  # padded conv width

    alpha_enc = _build.alpha_enc
    alpha_int = _build.alpha_int

    # packed-tensor column offsets (fp32 cols)
    # pack128: gx | bj | linbr | lowb | lobb
    o_bj = 0
    o_lbr = o_bj + njt
    o_low = o_lbr + 2 * NF
    o_lob = o_low + C * NS * 2 * C * NBASIS  # loBs: (c, s, d, k)
    n128 = o_lob + NTT * W24
    # pack32: gwab | gbn | w1 | w2 | w3 | b123 | linw
    o_gbn = 2 * RIN
    o_w1 = o_gbn + 1
    o_w2 = o_w1 + KW * ROUT
    o_w3 = o_w2 + KW * ROUT
    o_b123 = o_w3 + KW * ROUT
    o_linw = o_b123 + 3
    n32 = o_linw + 2 * NF
    # packb: xtr | epsb ; xg pack: xs (both b) | gx
    o_xtr = 0
    o_eps = o_xtr + C * NTAR
    nb_ = o_eps + NF * NS
    o_gx = NBL * NCH * C
    nxg = o_gx + mext

    nc = bacc.Bacc("TRN2", target_bir_lowering=False, debug=False)

    d_xg = nc.dram_tensor("xg", [128, nxg], F32, kind="ExternalInput")
    d_p128 = nc.dram_tensor("p128", [128, n128], F32, kind="ExternalInput")
    d_p32 = nc.dram_tensor("p32", [ROUT, n32], F32, kind="ExternalInput")
    d_ion = nc.dram_tensor("ion", [1, 2 + mext], BF16, kind="ExternalInput")
    d_pb = nc.dram_tensor("pb", [NBL, 128, nb_], F32, kind="ExternalInput")
    d_yi = nc.dram_tensor("yi", [NBL, 128, NCH * C * 2], BF16, kind="ExternalInput")
    d_out = nc.dram_tensor("out", [NS, NBL, NTAR, 2 * C], F32, kind="ExternalOutput")

    with tile.TileContext(nc) as tc:
        import contextlib

        est = contextlib.ExitStack()
        with est:
            p_cst = est.enter_context(tc.tile_pool(name="cst", bufs=1))
            p_io = est.enter_context(tc.tile_pool(name="io", bufs=1))
            p_tab = est.enter_context(tc.tile_pool(name="tab", bufs=3))
            p_ei = est.enter_context(tc.tile_pool(name="ei", bufs=2 * njt))
            p_h = est.enter_context(tc.tile_pool(name="h", bufs=2))
            p_sm = est.enter_context(tc.tile_pool(name="sm", bufs=2))
            p_z = est.enter_context(tc.tile_pool(name="z", bufs=2))
            p_zz2 = est.enter_context(tc.tile_pool(name="zz2", bufs=2))
            ps_e = est.enter_context(tc.tile_pool(name="pse", bufs=3, space="PSUM"))
            ps_r = est.enter_context(tc.tile_pool(name="psr", bufs=1, space="PSUM"))
            ps_c = est.enter_context(tc.tile_pool(name="psc", bufs=2, space="PSUM"))
            ps_h = est.enter_context(tc.tile_pool(name="psh", bufs=1, space="PSUM"))
            ps_o = est.enter_context(tc.tile_pool(name="pso", bufs=1, space="PSUM"))

            # ---- input DMAs: per-batch packs first (encoder head), then
            # constants ----
            pbs, yis = [], []
            xg = p_cst.tile([128, nxg], F32)
            nc.sync.dma_start(xg[:], d_xg.ap())
            for b in range(NBL):
                pb = p_io.tile([128, nb_], F32, tag=f"pb{b}")
                nc.sync.dma_start(pb[:], d_pb.ap()[b])
                pbs.append(pb)
            p128 = p_cst.tile([128, n128], F32)
            nc.sync.dma_start(p128[:], d_p128.ap())
            for b in range(NBL):
                yi = p_io.tile([128, NCH * C * 2], BF16, tag=f"yi{b}")
                nc.sync.dma_start(yi[:], d_yi.ap()[b])
                yis.append(yi)
            ion = p_cst.tile([1, 2 + mext], BF16)
            nc.sync.dma_start(ion[:], d_ion.ap())
            p32 = p_cst.tile([ROUT, n32], F32R)
            nc.sync.dma_start(p32[:], d_p32.ap().bitcast(F32R))

            bj = p128[:, o_bj : o_bj + njt]
            linbr = p128[:, o_lbr : o_lbr + 2 * NF]
            loBs = p128[:, o_low : o_low + C * NS * 2 * C * NBASIS]
            lobb = p128[:, o_lob : o_lob + NTT * W24]
            gwab = p32[0:C, 0 : 2 * RIN]
            gbn = p32[0:RIN, o_gbn : o_gbn + 1].bitcast(F32)
            w1 = p32[0:RIN, o_w1 : o_w1 + KW * ROUT]
            w2 = p32[0:ROUT, o_w2 : o_w2 + KW * ROUT]
            w3 = p32[0:ROUT, o_w3 : o_w3 + KW * ROUT]
            b123 = p32[0:ROUT, o_b123 : o_b123 + 3].bitcast(F32)
            linw = p32[0:ROUT, o_linw : o_linw + 2 * NF]

            # conv act tiles (dedicated; pads zeroed once on Pool) + merged ot
            h0cs = [
                p_cst.tile([RIN, mp], F32R, name=f"h0c{i}") for i in range(NBL)
            ]
            h1cs = [
                p_cst.tile([ROUT, mp], F32R, name=f"h1c{i}") for i in range(NBL)
            ]
            h2cs = [
                p_cst.tile([ROUT, mp], F32R, name=f"h2c{i}") for i in range(NBL)
            ]
            for t in h0cs + h1cs + h2cs:
                nc.gpsimd.memset(t[:, 0:2].bitcast(F32), 0.0)
                nc.gpsimd.memset(t[:, 2 + m : mp].bitcast(F32), 0.0)
            ot = p_cst.tile([128, NBL * NTT * W24], F32)  # (b, tt, s, d)

            # ---- encoder tables (DErf, ACT queue head) + accumulation ----
            # d6[p, (ch, r)] = gx[aoff + q*ch + r] - xs[p, (ch, c)]
            gap = xg[:]
            win = AP(
                gap.tensor, gap.offset + o_gx + aoff,
                [list(gap.ap[0]), [q, NCH], [1, wf]],
            )
            psums = [[None] * C for _ in range(NBL)]
            for b in range(NBL):
                for c in range(C):
                    d6 = p_tab.tile([128, NCH * wf], F32, tag="d6")
                    xv = (
                        xg[:, b * NCH * C : (b + 1) * NCH * C]
                        .rearrange("p (ch c) -> p ch c", ch=NCH, c=C)[:, :, c]
                        .unsqueeze(2)
                        .broadcast_to([128, NCH, wf])
                    )
                    d6v = d6[:].rearrange("p (ch r) -> p ch r", ch=NCH, r=wf)
                    eng = nc.vector if b == 0 else nc.gpsimd
                    eng.tensor_tensor(d6v, win, xv, op=ALU.subtract)
                    e6 = p_tab.tile([128, NCH * wf], BF16, tag="e6")
                    last_e6 = nc.scalar.activation(
                        e6[:], d6[:], AF.Derivative_Erf, scale=float(alpha_enc[0])
                    )
                    ps2 = ps_e.tile([2, mext], F32, tag="pse")
                    nc.tensor.matmul(
                        ps2[:], ion[0:1, 0:2], ion[0:1, 2:],
                        start=True, stop=False, skip_group_check=True,
                    )
                    for ch in range(NCH):
                        s0 = aoff + q * ch
                        nc.tensor.matmul(
                            ps2[:, s0 : s0 + wf],
                            yis[b][:, (ch * C + c) * 2 : (ch * C + c) * 2 + 2],
                            e6[:, ch * wf : (ch + 1) * wf],
                            start=False, stop=(ch == NCH - 1),
                            skip_group_check=True,
                        )
                    psums[b][c] = ps2

            # ---- psum -> staging, relocation, bridge ----
            fH0s, nh3s = [], []
            for b in range(NBL):
                hcat = p_h.tile([2, C * m], F32R, tag="hcat")
                for c in range(C):
                    nc.vector.tensor_copy(
                        hcat[:, c * m : (c + 1) * m].bitcast(F32),
                        psums[b][c][:, moff : moff + m],
                    )
                fH0 = p_h.tile([C, m], F32R, tag="fH0")
                fH1 = p_h.tile([C, m], F32R, tag="fH1")
                nc.sync.dma_start(
                    fH0[:], hcat[0:1].rearrange("one (c m) -> one c m", c=C, m=m)
                )
                nc.sync.dma_start(
                    fH1[:], hcat[1:2].rearrange("one (c m) -> one c m", c=C, m=m)
                )
                rec3 = p_h.tile([C, m], F32, tag="rec3")
                nc.vector.reciprocal_approx_fast(rec3[:], fH0[:].bitcast(F32))
                nh3 = p_h.tile([C, m], F32R, tag="nh3")
                nc.vector.tensor_tensor(
                    nh3[:], fH1[:].bitcast(F32), rec3[:], op=ALU.mult
                )
                fH0s.append(fH0)
                nh3s.append(nh3)

            # ---- interp gaussian tables (still DErf; after all E6) ----
            eis = [[], []]
            for b in range(NBL):
                for jt in range(njt):
                    jts = mts[jt]
                    ei = p_ei.tile([128, C * NTAR], F32, tag="ei")
                    ai = nc.scalar.activation(
                        ei[:jts],
                        pbs[b][:jts, o_xtr : o_xtr + C * NTAR],
                        AF.Derivative_Erf,
                        bias=bj[:jts, jt : jt + 1], scale=float(alpha_int),
                    )
                    add_dep_helper(ai.ins, last_e6.ins, sync=False)
                    eis[b].append(ei)

            # ---- phase B (sigmoid table), batch-interleaved by stage ----
            h3s = [None, None]
            for b in range(NBL):
                rp = ps_r.tile([RIN, m], F32, tag="rp")
                nc.tensor.matmul(rp[:], gwab[:, :RIN], fH0s[b][:],
                                 start=True, stop=False, skip_group_check=True)
                nc.tensor.matmul(rp[:], gwab[:, RIN:], nh3s[b][:],
                                 start=False, stop=True, skip_group_check=True)
                nc.scalar.activation(
                    h0cs[b][:, 2 : 2 + m], rp[:], AF.Sigmoid, bias=gbn, scale=1.0
                )
            hins = [h0cs[0], h0cs[1]]
            for li, (wt, cin) in enumerate([(w1, RIN), (w2, ROUT), (w3, ROUT)]):
                for b in range(NBL):
                    cps = ps_c.tile([ROUT, m], F32, tag="cps")
                    for dk in range(KW):
                        nc.tensor.matmul(
                            cps[:], wt[:cin, dk * ROUT : (dk + 1) * ROUT],
                            hins[b][:cin, dk : dk + m],
                            start=(dk == 0), stop=(dk == KW - 1),
                        )
                    if li == 0:
                        nc.scalar.activation(
                            h1cs[b][:, 2 : 2 + m], cps[:], AF.Relu,
                            bias=b123[:, 0:1], scale=1.0,
                        )
                        hins[b] = h1cs[b]
                    elif li == 1:
                        nc.scalar.activation(
                            h2cs[b][:, 2 : 2 + m], cps[:], AF.Relu,
                            bias=b123[:, 1:2], scale=1.0,
                        )
                        hins[b] = h2cs[b]
                    else:
                        h3 = p_h.tile([ROUT, m], F32R, tag="h3")
                        nc.vector.tensor_scalar_add(h3[:], cps[:], b123[:, 2:3])
                        h3s[b] = h3

            # h_grid -> z (mu/sigma in split halves, (jt, c, k) order)
            zs = [None, None]
            hsigs = []
            for b in range(NBL):
                hgps = ps_h.tile([128, njt * 2 * NF], F32, tag="hgps")
                for jt in range(njt):
                    jts = mts[jt]
                    j0 = jt * 128
                    nc.tensor.matmul(
                        hgps[:jts, jt * 2 * NF : (jt + 1) * 2 * NF],
                        h3s[b][:, j0 : j0 + jts], linw,
                        start=True, stop=True, skip_group_check=True,
                    )
                hgsb = p_sm.tile([128, 2 * njt * NF], F32, tag="hgsb")
                nc.vector.tensor_tensor(
                    hgsb[:].rearrange(
                        "p (h jt t) -> p h jt t", h=2, jt=njt, t=NF
                    ),
                    hgps[:].rearrange(
                        "p (jt h t) -> p h jt t", jt=njt, h=2, t=NF
                    ),
                    linbr.rearrange("p (h t) -> p h t", h=2, t=NF)
                    .unsqueeze(2)
                    .broadcast_to([128, 2, njt, NF]),
                    op=ALU.add,
                )
                hs = p_sm.tile([128, njt * NF], F32, tag="hs")
                hsig = nc.scalar.activation(
                    hs[:], hgsb[:, njt * NF :], AF.Sigmoid
                )
                hsigs.append(hsig)
                nc.vector.tensor_scalar(
                    hs[:], hs[:], 0.9, 0.1, op0=ALU.mult, op1=ALU.add
                )
                z = p_z.tile([128, njt * NF * NS], F32, tag="z")
                zv = z[:].rearrange(
                    "p (jt c s k) -> p jt c s k", jt=njt, c=C, s=NS, k=NBASIS
                )
                hsv = (
                    hs[:]
                    .rearrange("p (jt c k) -> p jt c k", jt=njt, c=C, k=NBASIS)
                    .unsqueeze(3)
                    .broadcast_to([128, njt, C, NS, NBASIS])
                )
                ev = (
                    pbs[b][:, o_eps : o_eps + NF * NS]
                    .rearrange("p (k c s) -> p c s k", k=NBASIS, c=C, s=NS)
                    .unsqueeze(1)
                    .broadcast_to([128, njt, C, NS, NBASIS])
                )
                nc.vector.tensor_tensor(zv, hsv, ev, op=ALU.mult)
                muv = (
                    hgsb[:, : njt * NF]
                    .rearrange("p (jt c k) -> p jt c k", jt=njt, c=C, k=NBASIS)
                    .unsqueeze(3)
                    .broadcast_to([128, njt, C, NS, NBASIS])
                )
                nc.vector.tensor_tensor(zv, zv, muv, op=ALU.add)
                zs[b] = z

            # interp matmuls + per-batch softplus + out (one id6 table load
            # covers Abs/Relu/Exp/Ln for both batches)
            ld = mybir.InstLoadActFuncSet(
                name=nc.get_next_instruction_name(), ins=[], outs=[],
                act_func_set_id=6,
            )
            nc.scalar.add_instruction(ld)
            for hsig in hsigs:
                add_dep_helper(ld, hsig.ins, sync=False)
            nsk = NS * NBASIS
            for b in range(NBL):
                for tt in range(NTT):
                    # P[t, (c, s, k)] = sum_j ei_c[j, t] * z[j, (c, s, k)]
                    # P[t, (c, s, k)] accumulated over grid tiles
                    po = ps_o.tile([128, C * nsk], F32, tag="po")
                    for c in range(C):
                        t0 = c * NTAR + tt * 128
                        for jt in range(njt):
                            jts = mts[jt]
                            nc.tensor.matmul(
                                po[:, c * nsk : (c + 1) * nsk],
                                eis[b][jt][:jts, t0 : t0 + 128],
                                zs[b][
                                    :jts,
                                    jt * C * nsk + c * nsk : jt * C * nsk
                                    + (c + 1) * nsk,
                                ],
                                start=(jt == 0), stop=(jt == njt - 1),
                                skip_group_check=True,
                            )
                    # zz1[(c,s,d)] = sum_k P[(c,s,k)] * loBs[(c,s,d,k)]
                    zzt = p_sm.tile([128, C * NS * 2 * C * NBASIS], F32, tag="zzt")
                    zztv = zzt[:].rearrange(
                        "p (cs d k) -> p cs d k", cs=C * NS, d=2 * C, k=NBASIS
                    )
                    pv = (
                        po[:]
                        .rearrange("p (cs k) -> p cs k", cs=C * NS, k=NBASIS)
                        .unsqueeze(2)
                        .broadcast_to([128, C * NS, 2 * C, NBASIS])
                    )
                    lov = loBs.rearrange(
                        "p (cs d k) -> p cs d k", cs=C * NS, d=2 * C, k=NBASIS
                    )
                    nc.vector.tensor_tensor(zztv, pv, lov, op=ALU.mult)
                    zz1 = p_sm.tile([128, C * NS * 2 * C], F32, tag="zz1")
                    nc.vector.reduce_sum(
                        zz1[:],
                        zzt[:].rearrange(
                            "p (csd k) -> p csd k", csd=C * NS * 2 * C, k=NBASIS
                        ),
                        axis=mybir.AxisListType.X,
                    )
                    osl = ot[:, (b * NTT + tt) * W24 : (b * NTT + tt + 1) * W24]
                    nc.vector.tensor_tensor(
                        osl, zz1[:, 0 : W24], zz1[:, W24 : 2 * W24], op=ALU.add
                    )
                    nc.vector.tensor_tensor(osl, osl, zz1[:, 2 * W24 :], op=ALU.add)
                    nc.vector.tensor_tensor(
                        osl, osl, lobb[:, tt * W24 : (tt + 1) * W24], op=ALU.add
                    )
                # softplus on this batch's std cols
                ng = NTT * NS
                sv = ot[:, b * NTT * W24 : (b + 1) * NTT * W24].rearrange(
                    "p (g d) -> p g d", g=ng, d=2 * C
                )[:, :, C:]
                av = p_sm.tile([128, ng * C], F32, tag="av")
                avv = av[:].rearrange("p (g d) -> p g d", g=ng, d=C)
                a1 = nc.scalar.activation(avv, sv, AF.Abs)
                add_dep_helper(a1.ins, ld, sync=False)
                rv = p_sm.tile([128, ng * C], F32, tag="rv")
                rvv = rv[:].rearrange("p (g d) -> p g d", g=ng, d=C)
                a2 = nc.scalar.activation(rvv, sv, AF.Relu)
                add_dep_helper(a2.ins, ld, sync=False)
                ew = p_sm.tile([128, ng * C], F32, tag="ew")
                a3 = nc.scalar.activation(ew[:], av[:], AF.Exp, scale=-1.0)
                add_dep_helper(a3.ins, ld, sync=False)
                lw_ = p_sm.tile([128, ng * C], F32, tag="lw_")
                a4 = nc.scalar.activation(lw_[:], ew[:], AF.Ln, bias=1.0)
                add_dep_helper(a4.ins, ld, sync=False)
                lvv = lw_[:].rearrange("p (g d) -> p g d", g=ng, d=C)
                nc.vector.tensor_tensor(sv, rvv, lvv, op=ALU.add)
                for tt in range(NTT):
                    nc.sync.dma_start(
                        d_out.ap()[:, b, tt * 128 : (tt + 1) * 128, :].rearrange(
                            "s p d -> p s d"
                        ),
                        ot[
                            :, (b * NTT + tt) * W24 : (b * NTT + tt + 1) * W24
                        ].rearrange("p (s d) -> p s d", s=NS, d=2 * C),
                    )

    nc.compile()
    return nc


def _prep(inputs):
    """Host-side sorting/pair-merging/packing. Returns (key, per-core in_maps)."""
    x = np.ascontiguousarray(inputs["x"], dtype=np.float32)
    y = np.ascontiguousarray(inputs["y"], dtype=np.float32)
    x_out = np.ascontiguousarray(inputs["x_out"], dtype=np.float32)
    x_grid = np.asarray(inputs["x_grid"], dtype=np.float32)
    eps_noise = np.asarray(inputs["eps_noise"], dtype=np.float32)
    enc_sigma = np.asarray(inputs["enc_sigma"], dtype=np.float64)
    int_sigma = np.asarray(inputs["int_sigma"], dtype=np.float64)
    gW = np.asarray(inputs["gW"], dtype=np.float32)
    gb = np.asarray(inputs["gb"], dtype=np.float32)
    w1 = np.asarray(inputs["w1"], dtype=np.float32)
    b1 = np.asarray(inputs["b1"], dtype=np.float32)
    w2 = np.asarray(inputs["w2"], dtype=np.float32)
    b2 = np.asarray(inputs["b2"], dtype=np.float32)
    w3 = np.asarray(inputs["w3"], dtype=np.float32)
    b3 = np.asarray(inputs["b3"], dtype=np.float32)
    linW = np.asarray(inputs["linW"], dtype=np.float32)
    linb = np.asarray(inputs["linb"], dtype=np.float32)
    loW = np.asarray(inputs["loW"], dtype=np.float32)
    lob = np.asarray(inputs["lob"], dtype=np.float32)

    nb, npts, _ = x.shape
    assert nb == NB and npts == NPTS
    m = x_grid.shape[1]
    njt = (m + 127) // 128
    g = x_grid[0, :, 0].astype(np.float64)
    g0 = float(g[0])
    gd = float((g[-1] - g[0]) / (m - 1))

    s_enc = np.exp(enc_sigma) + EPS
    alpha_enc = 1.0 / (np.sqrt(2.0) * s_enc)
    assert np.ptp(alpha_enc) < 1e-9 * abs(alpha_enc[0]), "enc_sigma must be uniform"
    s_int = np.exp(int_sigma) + EPS
    assert np.ptp(s_int) < 1e-12 * abs(s_int.flat[0]), "int_sigma must be uniform"
    alpha_int = float(1.0 / (np.sqrt(2.0) * s_int.flat[0]))
    _build.alpha_enc = [float(a) for a in alpha_enc]
    _build.alpha_int = alpha_int

    # ---- sort + pair-merge points per (b, c); global affine window lattice
    srt = np.sort(x.transpose(0, 2, 1), axis=2)
    idx = np.argsort(x.transpose(0, 2, 1), axis=2, kind="stable")
    ysrt = np.take_along_axis(y.transpose(0, 2, 1), idx, axis=2)
    xs_all = 0.5 * (srt[:, :, 0::2] + srt[:, :, 1::2])
    ys_all = ysrt[:, :, 0::2] + ysrt[:, :, 1::2]
    chunks = xs_all.reshape(NB, C, NCH, 128)
    reach = KREACH / alpha_enc.reshape(1, 3, 1)
    c_lo = np.ceil((chunks[:, :, :, 0] - reach - g0) / gd).astype(int)
    c_hi = np.floor((chunks[:, :, :, -1] + reach - g0) / gd).astype(int)
    ch_idx = np.arange(NCH)
    qfit = (c_lo[:, :, -1] + c_hi[:, :, -1] - c_lo[:, :, 0] - c_hi[:, :, 0]) / (
        2.0 * (NCH - 1)
    )
    q = int(round(float(np.median(qfit))))
    a = int((c_lo - q * ch_idx).min())
    whi = int((c_hi - q * ch_idx).max())
    wf = whi - a + 1
    off = min(a, 0)
    aoff = a - off
    mext = max(m, a + q * (NCH - 1) + wf) - off
    assert mext <= 512, f"psum extent {mext} > 512"
    assert wf <= 128, f"window {wf} too wide"

    # ---- packed device tensors ----
    o_bj = 0
    o_lbr = o_bj + njt
    o_low = o_lbr + 2 * NF
    o_lob = o_low + C * NS * 2 * C * NBASIS
    n128 = o_lob + NTT * W24
    o_gbn = 2 * RIN
    o_w1 = o_gbn + 1
    o_w2 = o_w1 + KW * ROUT
    o_w3 = o_w2 + KW * ROUT
    o_b123 = o_w3 + KW * ROUT
    o_linw = o_b123 + 3
    n32 = o_linw + 2 * NF
    o_xtr = 0
    o_eps = o_xtr + C * NTAR
    nb_ = o_eps + NF * NS
    o_gx = NBL * NCH * C

    p128 = np.zeros((128, n128), np.float32)
    gpad = np.zeros(njt * 128, np.float32)
    gpad[:m] = g.astype(np.float32)
    p128[:, o_bj : o_bj + njt] = (-alpha_int * gpad).reshape(njt, 128).T
    perm = np.array(
        [h * 15 + k * C + c for h in range(2) for c in range(C) for k in range(NBASIS)]
    )
    p128[:, o_lbr : o_lbr + 2 * NF] = linb[perm][None, :]
    lo = KAPPA * loW.reshape(NBASIS, C, 2 * C)  # (k, c, d)
    loBs_vec = (
        np.broadcast_to(lo.transpose(1, 2, 0)[:, None, :, :], (C, NS, 2 * C, NBASIS))
        .reshape(C * NS * 2 * C * NBASIS)
        .astype(np.float32)
    )
    p128[:, o_low : o_low + C * NS * 2 * C * NBASIS] = loBs_vec[None, :]
    p128[:, o_lob : o_lob + NTT * W24] = np.tile(lob, NTT * NS)[None, :]

    p32 = np.zeros((ROUT, n32), np.float32)
    p32[0:C, 0 : 2 * RIN] = np.concatenate([KAPPA * gW[0:3], gW[3:6]], axis=1)
    p32[0:RIN, o_gbn] = gb
    p32[0:RIN, o_w1 : o_w1 + KW * ROUT] = w1.transpose(1, 2, 0).reshape(RIN, -1)
    p32[0:ROUT, o_w2 : o_w2 + KW * ROUT] = w2.transpose(1, 2, 0).reshape(ROUT, -1)
    p32[0:ROUT, o_w3 : o_w3 + KW * ROUT] = w3.transpose(1, 2, 0).reshape(ROUT, -1)
    p32[0:ROUT, o_b123 : o_b123 + 3] = np.stack([b1, b2, b3], axis=1)
    p32[0:ROUT, o_linw : o_linw + 2 * NF] = linW[:, perm]

    ion = np.zeros((1, 2 + mext), np.float32)
    ion[0, 0] = EPS / KAPPA
    ion[0, 2:] = 1.0
    ion = ion.astype(ml_dtypes.bfloat16)

    xsr = xs_all.reshape(NB, C, NCH, 128).transpose(0, 3, 2, 1).reshape(NB, 128, -1)
    gxrow = (g0 + gd * (np.arange(mext) + off)).astype(np.float32)
    pball = np.empty((NB, 128, nb_), np.float32)
    pball[:, :, o_xtr : o_xtr + C * NTAR] = np.broadcast_to(
        x_out.transpose(0, 2, 1).reshape(NB, 1, C * NTAR), (NB, 128, C * NTAR)
    )
    pball[:, :, o_eps :] = np.broadcast_to(
        eps_noise.transpose(1, 2, 0).reshape(NB, 1, NF * NS), (NB, 128, NF * NS)
    )
    yi = np.empty((NB, 128, NCH * C * 2), np.float32)
    yi[:, :, 0::2] = 2.0
    yi[:, :, 1::2] = (
        ys_all.reshape(NB, C, NCH, 128).transpose(0, 3, 2, 1).reshape(NB, 128, -1)
    )
    yi = yi.astype(ml_dtypes.bfloat16)

    in_maps = []
    for core in range(NCORES):
        bsl = slice(core * NBL, (core + 1) * NBL)
        xgc = np.empty((128, NBL * NCH * C + mext), np.float32)
        for bl in range(NBL):
            xgc[:, bl * NCH * C : (bl + 1) * NCH * C] = xsr[core * NBL + bl]
        xgc[:, NBL * NCH * C :] = gxrow[None, :]
        in_maps.append(
            {
                "xg": xgc,
                "p128": p128,
                "p32": p32,
                "ion": ion,
                "pb": pball[bsl].copy(),
                "yi": np.ascontiguousarray(yi[bsl]),
            }
        )
    key = (m, q, aoff, wf, mext, -off, _build.alpha_int, tuple(_build.alpha_enc))
    return key, in_maps


def kernel(**inputs):
    key, in_maps = _prep(inputs)
    if key not in _CACHE:
        _CACHE[key] = _build(*key[:6])
    nc = _CACHE[key]
    res = bass_utils.run_bass_kernel_spmd(nc, in_maps, core_ids=list(range(NCORES)))
    outs = [res.results[c]["out"] for c in range(NCORES)]  # each (NS, NBL, NTAR, 6)
    full = np.concatenate(outs, axis=1)  # (NS, NB, NTAR, 6)
    return full.astype(np.float32)


# revision 29
# speedup vs baseline: 1.0925x; 1.0925x over previous
"""Trainium2 Bass kernel for a latent ConvCNP (gaussian encoder -> CNN ->
latent samples -> gaussian interpolator), data-parallel over batch on 8
NeuronCores.

v5: sorted + pair-merged windowed encoder (see v4 notes) with packed input
DMAs (3 constant packs + 2 per-batch packs instead of 21 transfers),
per-channel table pipelining, a single merged h_grid psum + add, a
manually placed natural_log_exp act-table load (3 table loads total), and
per-batch softplus + output DMAs so batch 0's results leave the core while
batch 1 is still in flight.

Contract: kernel(**inputs) takes the full unsharded inputs (numpy) and
returns the full (NS, nb, ntar, 2C) output.
"""

import sys

sys.path.insert(0, "/opt/trn_rl_repo")

import math

import ml_dtypes
import numpy as np

import concourse.bacc as bacc
import concourse.mybir as mybir
import concourse.tile as tile
from concourse import bass_utils
from concourse.ap import AP
from concourse.tile_rust import add_dep_helper

F32 = mybir.dt.float32
F32R = mybir.dt.float32r
BF16 = mybir.dt.bfloat16
AF = mybir.ActivationFunctionType
ALU = mybir.AluOpType

# problem constants (fixed by the reference problem)
EPS = 1e-6
C = 3
NBASIS = 5
NS = 4
RIN = 16
ROUT = 32
KW = 5
NB = 16          # full batch
NPTS = 2048
NTAR = 256
NCORES = 8
NBL = NB // NCORES   # batches per core
NPM = NPTS // 2      # pair-merged points
NCH = NPM // 128     # 8 chunks per (b, c)
KAPPA = math.sqrt(math.pi) / 2.0  # exp(-x^2) = KAPPA * Derivative_Erf(x)
KREACH = 4.0                      # window reach in units of 1/alpha
NF = C * NBASIS
NTT = NTAR // 128
W24 = NS * 2 * C

_CACHE = {}


def _build(m, q, aoff, wf, mext, moff):
    """Per-core Bass program. m = grid cols; chunk ch's window occupies psum
    cols [aoff + q*ch, +wf); grid col j lives at psum col j + moff (the gx
    input content is shifted to match)."""
    njt = (m + 127) // 128
    mts = [128] * (m // 128) + ([m % 128] if m % 128 else [])
    mp = m + 4  # padded conv width

    alpha_enc = _build.alpha_enc
    alpha_int = _build.alpha_int

    # packed-tensor column offsets (fp32 cols)
    # pack128: gx | bj | linbr | lowb | lobb
    o_bj = 0
    o_lbr = o_bj + njt
    o_low = o_lbr + 2 * NF
    o_lob = o_low + C * NS * 2 * C * NBASIS  # loBs: (c, s, d, k)
    n128 = o_lob + NTT * W24
    # pack32: gwab | gbn | w1 | w2 | w3 | b123 | linw
    o_gbn = 2 * RIN
    o_w1 = o_gbn + 1
    o_w2 = o_w1 + KW * ROUT
    o_w3 = o_w2 + KW * ROUT
    o_b123 = o_w3 + KW * ROUT
    o_linw = o_b123 + 3
    n32 = o_linw + 2 * NF
    # packb: xtr | epsb ; xg pack: xs (both b) | gx
    o_xtr = 0
    o_eps = o_xtr + C * NTAR
    nb_ = o_eps + NF * NS
    o_gx = NBL * NCH * C
    nxg = o_gx + mext

    nc = bacc.Bacc("TRN2", target_bir_lowering=False, debug=False)

    d_xg = nc.dram_tensor("xg", [128, nxg], F32, kind="ExternalInput")
    d_p128 = nc.dram_tensor("p128", [128, n128], F32, kind="ExternalInput")
    d_p32 = nc.dram_tensor("p32", [ROUT, n32], F32, kind="ExternalInput")
    d_ion = nc.dram_tensor("ion", [1, 2 + mext], BF16, kind="ExternalInput")
    d_pb = nc.dram_tensor("pb", [NBL, 128, nb_], F32, kind="ExternalInput")
    d_yi = nc.dram_tensor("yi", [NBL, 128, NCH * C * 2], BF16, kind="ExternalInput")
    d_out = nc.dram_tensor("out", [NS, NBL, NTAR, 2 * C], F32, kind="ExternalOutput")

    with tile.TileContext(nc) as tc:
        import contextlib

        est = contextlib.ExitStack()
        with est:
            p_cst = est.enter_context(tc.tile_pool(name="cst", bufs=1))
            p_io = est.enter_context(tc.tile_pool(name="io", bufs=1))
            p_tab = est.enter_context(tc.tile_pool(name="tab", bufs=3))
            p_ei = est.enter_context(tc.tile_pool(name="ei", bufs=2 * njt))
            p_h = est.enter_context(tc.tile_pool(name="h", bufs=2))
            p_sm = est.enter_context(tc.tile_pool(name="sm", bufs=2))
            p_z = est.enter_context(tc.tile_pool(name="z", bufs=2))
            p_zz2 = est.enter_context(tc.tile_pool(name="zz2", bufs=2))
            ps_e = est.enter_context(tc.tile_pool(name="pse", bufs=3, space="PSUM"))
            ps_r = est.enter_context(tc.tile_pool(name="psr", bufs=1, space="PSUM"))
            ps_c = est.enter_context(tc.tile_pool(name="psc", bufs=2, space="PSUM"))
            ps_h = est.enter_context(tc.tile_pool(name="psh", bufs=1, space="PSUM"))
            ps_o = est.enter_context(tc.tile_pool(name="pso", bufs=1, space="PSUM"))

            # ---- input DMAs: per-batch packs first (encoder head), then
            # constants ----
            pbs, yis = [], []
            ion = p_cst.tile([1, 2 + mext], BF16)
            nc.sync.dma_start(ion[:], d_ion.ap())
            xg = p_cst.tile([128, nxg], F32)
            nc.sync.dma_start(xg[:], d_xg.ap())
            for b in range(NBL):
                pb = p_io.tile([128, nb_], F32, tag=f"pb{b}")
                nc.sync.dma_start(pb[:], d_pb.ap()[b])
                pbs.append(pb)
            p128 = p_cst.tile([128, n128], F32)
            nc.sync.dma_start(p128[:], d_p128.ap())
            for b in range(NBL):
                yi = p_io.tile([128, NCH * C * 2], BF16, tag=f"yi{b}")
                nc.sync.dma_start(yi[:], d_yi.ap()[b])
                yis.append(yi)
            p32 = p_cst.tile([ROUT, n32], F32R)
            nc.sync.dma_start(p32[:], d_p32.ap().bitcast(F32R))

            bj = p128[:, o_bj : o_bj + njt]
            linbr = p128[:, o_lbr : o_lbr + 2 * NF]
            loBs = p128[:, o_low : o_low + C * NS * 2 * C * NBASIS]
            lobb = p128[:, o_lob : o_lob + NTT * W24]
            gwab = p32[0:C, 0 : 2 * RIN]
            gbn = p32[0:RIN, o_gbn : o_gbn + 1].bitcast(F32)
            w1 = p32[0:RIN, o_w1 : o_w1 + KW * ROUT]
            w2 = p32[0:ROUT, o_w2 : o_w2 + KW * ROUT]
            w3 = p32[0:ROUT, o_w3 : o_w3 + KW * ROUT]
            b123 = p32[0:ROUT, o_b123 : o_b123 + 3].bitcast(F32)
            linw = p32[0:ROUT, o_linw : o_linw + 2 * NF]

            # conv act tiles (dedicated; pads zeroed once on Pool) + merged ot
            h0cs = [
                p_cst.tile([RIN, mp], F32R, name=f"h0c{i}") for i in range(NBL)
            ]
            h1cs = [
                p_cst.tile([ROUT, mp], F32R, name=f"h1c{i}") for i in range(NBL)
            ]
            h2cs = [
                p_cst.tile([ROUT, mp], F32R, name=f"h2c{i}") for i in range(NBL)
            ]
            for t in h0cs + h1cs + h2cs:
                nc.gpsimd.memset(t[:, 0:2].bitcast(F32), 0.0)
                nc.gpsimd.memset(t[:, 2 + m : mp].bitcast(F32), 0.0)
            ot = p_cst.tile([128, NBL * NTT * W24], F32)  # (b, tt, s, d)

            # ---- encoder tables (DErf, ACT queue head) + accumulation ----
            # d6[p, (ch, r)] = gx[aoff + q*ch + r] - xs[p, (ch, c)]
            gap = xg[:]
            win = AP(
                gap.tensor, gap.offset + o_gx + aoff,
                [list(gap.ap[0]), [q, NCH], [1, wf]],
            )
            psums = [[None] * C for _ in range(NBL)]
            for b in range(NBL):
                for c in range(C):
                    d6 = p_tab.tile([128, NCH * wf], F32, tag="d6")
                    xv = (
                        xg[:, b * NCH * C : (b + 1) * NCH * C]
                        .rearrange("p (ch c) -> p ch c", ch=NCH, c=C)[:, :, c]
                        .unsqueeze(2)
                        .broadcast_to([128, NCH, wf])
                    )
                    d6v = d6[:].rearrange("p (ch r) -> p ch r", ch=NCH, r=wf)
                    nc.vector.tensor_tensor(d6v, win, xv, op=ALU.subtract)
                    e6 = p_tab.tile([128, NCH * wf], BF16, tag="e6")
                    nc.scalar.activation(
                        e6[:], d6[:], AF.Derivative_Erf, scale=float(alpha_enc[0])
                    )
                    ps2 = ps_e.tile([2, mext], F32, tag="pse")
                    nc.tensor.matmul(
                        ps2[:], ion[0:1, 0:2], ion[0:1, 2:],
                        start=True, stop=False, skip_group_check=True,
                    )
                    for ch in range(NCH):
                        s0 = aoff + q * ch
                        nc.tensor.matmul(
                            ps2[:, s0 : s0 + wf],
                            yis[b][:, (ch * C + c) * 2 : (ch * C + c) * 2 + 2],
                            e6[:, ch * wf : (ch + 1) * wf],
                            start=False, stop=(ch == NCH - 1),
                            skip_group_check=True,
                        )
                    psums[b][c] = ps2

            # ---- psum -> staging, relocation, bridge ----
            fH0s, nh3s = [], []
            for b in range(NBL):
                hcat = p_h.tile([2, C * m], F32R, tag="hcat")
                for c in range(C):
                    nc.vector.tensor_copy(
                        hcat[:, c * m : (c + 1) * m].bitcast(F32),
                        psums[b][c][:, moff : moff + m],
                    )
                fH0 = p_h.tile([C, m], F32R, tag="fH0")
                fH1 = p_h.tile([C, m], F32R, tag="fH1")
                nc.sync.dma_start(
                    fH0[:], hcat[0:1].rearrange("one (c m) -> one c m", c=C, m=m)
                )
                nc.sync.dma_start(
                    fH1[:], hcat[1:2].rearrange("one (c m) -> one c m", c=C, m=m)
                )
                rec3 = p_h.tile([C, m], F32, tag="rec3")
                nc.vector.reciprocal_approx_fast(rec3[:], fH0[:].bitcast(F32))
                nh3 = p_h.tile([C, m], F32R, tag="nh3")
                nc.vector.tensor_tensor(
                    nh3[:], fH1[:].bitcast(F32), rec3[:], op=ALU.mult
                )
                fH0s.append(fH0)
                nh3s.append(nh3)

            # ---- interp gaussian tables (still DErf; after all E6) ----
            eis = [[], []]
            for b in range(NBL):
                for jt in range(njt):
                    jts = mts[jt]
                    ei = p_ei.tile([128, C * NTAR], F32, tag="ei")
                    nc.scalar.activation(
                        ei[:jts],
                        pbs[b][:jts, o_xtr : o_xtr + C * NTAR],
                        AF.Derivative_Erf,
                        bias=bj[:jts, jt : jt + 1], scale=float(alpha_int),
                    )
                    eis[b].append(ei)

            # ---- phase B (sigmoid table), batch-interleaved by stage ----
            h3s = [None, None]
            for b in range(NBL):
                rp = ps_r.tile([RIN, m], F32, tag="rp")
                nc.tensor.matmul(rp[:], gwab[:, :RIN], fH0s[b][:],
                                 start=True, stop=False, skip_group_check=True)
                nc.tensor.matmul(rp[:], gwab[:, RIN:], nh3s[b][:],
                                 start=False, stop=True, skip_group_check=True)
                nc.scalar.activation(
                    h0cs[b][:, 2 : 2 + m], rp[:], AF.Sigmoid, bias=gbn, scale=1.0
                )
            hins = [h0cs[0], h0cs[1]]
            for li, (wt, cin) in enumerate([(w1, RIN), (w2, ROUT), (w3, ROUT)]):
                for b in range(NBL):
                    cps = ps_c.tile([ROUT, m], F32, tag="cps")
                    for dk in range(KW):
                        nc.tensor.matmul(
                            cps[:], wt[:cin, dk * ROUT : (dk + 1) * ROUT],
                            hins[b][:cin, dk : dk + m],
                            start=(dk == 0), stop=(dk == KW - 1),
                        )
                    if li == 0:
                        nc.scalar.activation(
                            h1cs[b][:, 2 : 2 + m], cps[:], AF.Relu,
                            bias=b123[:, 0:1], scale=1.0,
                        )
                        hins[b] = h1cs[b]
                    elif li == 1:
                        nc.scalar.activation(
                            h2cs[b][:, 2 : 2 + m], cps[:], AF.Relu,
                            bias=b123[:, 1:2], scale=1.0,
                        )
                        hins[b] = h2cs[b]
                    else:
                        h3 = p_h.tile([ROUT, m], F32R, tag="h3")
                        nc.vector.tensor_scalar_add(h3[:], cps[:], b123[:, 2:3])
                        h3s[b] = h3

            # h_grid -> z (mu/sigma in split halves, (jt, c, k) order)
            zs = [None, None]
            hsigs = []
            for b in range(NBL):
                hgps = ps_h.tile([128, njt * 2 * NF], F32, tag="hgps")
                for jt in range(njt):
                    jts = mts[jt]
                    j0 = jt * 128
                    nc.tensor.matmul(
                        hgps[:jts, jt * 2 * NF : (jt + 1) * 2 * NF],
                        h3s[b][:, j0 : j0 + jts], linw,
                        start=True, stop=True, skip_group_check=True,
                    )
                hgsb = p_sm.tile([128, 2 * njt * NF], F32, tag="hgsb")
                nc.vector.tensor_tensor(
                    hgsb[:].rearrange(
                        "p (h jt t) -> p h jt t", h=2, jt=njt, t=NF
                    ),
                    hgps[:].rearrange(
                        "p (jt h t) -> p h jt t", jt=njt, h=2, t=NF
                    ),
                    linbr.rearrange("p (h t) -> p h t", h=2, t=NF)
                    .unsqueeze(2)
                    .broadcast_to([128, 2, njt, NF]),
                    op=ALU.add,
                )
                hs = p_sm.tile([128, njt * NF], F32, tag="hs")
                hsig = nc.scalar.activation(
                    hs[:], hgsb[:, njt * NF :], AF.Sigmoid
                )
                hsigs.append(hsig)
                nc.vector.tensor_scalar(
                    hs[:], hs[:], 0.9, 0.1, op0=ALU.mult, op1=ALU.add
                )
                z = p_z.tile([128, njt * NF * NS], F32, tag="z")
                zv = z[:].rearrange(
                    "p (jt c s k) -> p jt c s k", jt=njt, c=C, s=NS, k=NBASIS
                )
                hsv = (
                    hs[:]
                    .rearrange("p (jt c k) -> p jt c k", jt=njt, c=C, k=NBASIS)
                    .unsqueeze(3)
                    .broadcast_to([128, njt, C, NS, NBASIS])
                )
                ev = (
                    pbs[b][:, o_eps : o_eps + NF * NS]
                    .rearrange("p (k c s) -> p c s k", k=NBASIS, c=C, s=NS)
                    .unsqueeze(1)
                    .broadcast_to([128, njt, C, NS, NBASIS])
                )
                nc.vector.tensor_tensor(zv, hsv, ev, op=ALU.mult)
                muv = (
                    hgsb[:, : njt * NF]
                    .rearrange("p (jt c k) -> p jt c k", jt=njt, c=C, k=NBASIS)
                    .unsqueeze(3)
                    .broadcast_to([128, njt, C, NS, NBASIS])
                )
                nc.vector.tensor_tensor(zv, zv, muv, op=ALU.add)
                zs[b] = z

            # interp matmuls + per-batch softplus + out (one id6 table load
            # covers Abs/Relu/Exp/Ln for both batches)
            ld = mybir.InstLoadActFuncSet(
                name=nc.get_next_instruction_name(), ins=[], outs=[],
                act_func_set_id=6,
            )
            nc.scalar.add_instruction(ld)
            for hsig in hsigs:
                add_dep_helper(ld, hsig.ins, sync=False)
            nsk = NS * NBASIS
            for b in range(NBL):
                for tt in range(NTT):
                    # P[t, (c, s, k)] = sum_j ei_c[j, t] * z[j, (c, s, k)]
                    # P[t, (c, s, k)] accumulated over grid tiles
                    po = ps_o.tile([128, C * nsk], F32, tag="po")
                    for c in range(C):
                        t0 = c * NTAR + tt * 128
                        for jt in range(njt):
                            jts = mts[jt]
                            nc.tensor.matmul(
                                po[:, c * nsk : (c + 1) * nsk],
                                eis[b][jt][:jts, t0 : t0 + 128],
                                zs[b][
                                    :jts,
                                    jt * C * nsk + c * nsk : jt * C * nsk
                                    + (c + 1) * nsk,
                                ],
                                start=(jt == 0), stop=(jt == njt - 1),
                                skip_group_check=True,
                            )
                    # zz1[(c,s,d)] = sum_k P[(c,s,k)] * loBs[(c,s,d,k)]
                    zzt = p_sm.tile([128, C * NS * 2 * C * NBASIS], F32, tag="zzt")
                    zztv = zzt[:].rearrange(
                        "p (cs d k) -> p cs d k", cs=C * NS, d=2 * C, k=NBASIS
                    )
                    pv = (
                        po[:]
                        .rearrange("p (cs k) -> p cs k", cs=C * NS, k=NBASIS)
                        .unsqueeze(2)
                        .broadcast_to([128, C * NS, 2 * C, NBASIS])
                    )
                    lov = loBs.rearrange(
                        "p (cs d k) -> p cs d k", cs=C * NS, d=2 * C, k=NBASIS
                    )
                    nc.vector.tensor_tensor(zztv, pv, lov, op=ALU.mult)
                    zz1 = p_sm.tile([128, C * NS * 2 * C], F32, tag="zz1")
                    nc.vector.reduce_sum(
                        zz1[:],
                        zzt[:].rearrange(
                            "p (csd k) -> p csd k", csd=C * NS * 2 * C, k=NBASIS
                        ),
                        axis=mybir.AxisListType.X,
                    )
                    osl = ot[:, (b * NTT + tt) * W24 : (b * NTT + tt + 1) * W24]
                    nc.vector.tensor_tensor(
                        osl, zz1[:, 0 : W24], zz1[:, W24 : 2 * W24], op=ALU.add
                    )
                    nc.vector.tensor_tensor(osl, osl, zz1[:, 2 * W24 :], op=ALU.add)
                    nc.vector.tensor_tensor(
                        osl, osl, lobb[:, tt * W24 : (tt + 1) * W24], op=ALU.add
                    )
                # softplus on this batch's std cols
                ng = NTT * NS
                sv = ot[:, b * NTT * W24 : (b + 1) * NTT * W24].rearrange(
                    "p (g d) -> p g d", g=ng, d=2 * C
                )[:, :, C:]
                av = p_sm.tile([128, ng * C], F32, tag="av")
                avv = av[:].rearrange("p (g d) -> p g d", g=ng, d=C)
                a1 = nc.scalar.activation(avv, sv, AF.Abs)
                add_dep_helper(a1.ins, ld, sync=False)
                rv = p_sm.tile([128, ng * C], F32, tag="rv")
                rvv = rv[:].rearrange("p (g d) -> p g d", g=ng, d=C)
                a2 = nc.scalar.activation(rvv, sv, AF.Relu)
                add_dep_helper(a2.ins, ld, sync=False)
                ew = p_sm.tile([128, ng * C], F32, tag="ew")
                a3 = nc.scalar.activation(ew[:], av[:], AF.Exp, scale=-1.0)
                add_dep_helper(a3.ins, ld, sync=False)
                lw_ = p_sm.tile([128, ng * C], F32, tag="lw_")
                a4 = nc.scalar.activation(lw_[:], ew[:], AF.Ln, bias=1.0)
                add_dep_helper(a4.ins, ld, sync=False)
                lvv = lw_[:].rearrange("p (g d) -> p g d", g=ng, d=C)
                nc.vector.tensor_tensor(sv, rvv, lvv, op=ALU.add)
                for tt in range(NTT):
                    nc.sync.dma_start(
                        d_out.ap()[:, b, tt * 128 : (tt + 1) * 128, :].rearrange(
                            "s p d -> p s d"
                        ),
                        ot[
                            :, (b * NTT + tt) * W24 : (b * NTT + tt + 1) * W24
                        ].rearrange("p (s d) -> p s d", s=NS, d=2 * C),
                    )

    nc.compile()
    return nc


def _prep(inputs):
    """Host-side sorting/pair-merging/packing. Returns (key, per-core in_maps)."""
    x = np.ascontiguousarray(inputs["x"], dtype=np.float32)
    y = np.ascontiguousarray(inputs["y"], dtype=np.float32)
    x_out = np.ascontiguousarray(inputs["x_out"], dtype=np.float32)
    x_grid = np.asarray(inputs["x_grid"], dtype=np.float32)
    eps_noise = np.asarray(inputs["eps_noise"], dtype=np.float32)
    enc_sigma = np.asarray(inputs["enc_sigma"], dtype=np.float64)
    int_sigma = np.asarray(inputs["int_sigma"], dtype=np.float64)
    gW = np.asarray(inputs["gW"], dtype=np.float32)
    gb = np.asarray(inputs["gb"], dtype=np.float32)
    w1 = np.asarray(inputs["w1"], dtype=np.float32)
    b1 = np.asarray(inputs["b1"], dtype=np.float32)
    w2 = np.asarray(inputs["w2"], dtype=np.float32)
    b2 = np.asarray(inputs["b2"], dtype=np.float32)
    w3 = np.asarray(inputs["w3"], dtype=np.float32)
    b3 = np.asarray(inputs["b3"], dtype=np.float32)
    linW = np.asarray(inputs["linW"], dtype=np.float32)
    linb = np.asarray(inputs["linb"], dtype=np.float32)
    loW = np.asarray(inputs["loW"], dtype=np.float32)
    lob = np.asarray(inputs["lob"], dtype=np.float32)

    nb, npts, _ = x.shape
    assert nb == NB and npts == NPTS
    m = x_grid.shape[1]
    njt = (m + 127) // 128
    g = x_grid[0, :, 0].astype(np.float64)
    g0 = float(g[0])
    gd = float((g[-1] - g[0]) / (m - 1))

    s_enc = np.exp(enc_sigma) + EPS
    alpha_enc = 1.0 / (np.sqrt(2.0) * s_enc)
    assert np.ptp(alpha_enc) < 1e-9 * abs(alpha_enc[0]), "enc_sigma must be uniform"
    s_int = np.exp(int_sigma) + EPS
    assert np.ptp(s_int) < 1e-12 * abs(s_int.flat[0]), "int_sigma must be uniform"
    alpha_int = float(1.0 / (np.sqrt(2.0) * s_int.flat[0]))
    _build.alpha_enc = [float(a) for a in alpha_enc]
    _build.alpha_int = alpha_int

    # ---- sort + pair-merge points per (b, c); global affine window lattice
    srt = np.sort(x.transpose(0, 2, 1), axis=2)
    idx = np.argsort(x.transpose(0, 2, 1), axis=2, kind="stable")
    ysrt = np.take_along_axis(y.transpose(0, 2, 1), idx, axis=2)
    xs_all = 0.5 * (srt[:, :, 0::2] + srt[:, :, 1::2])
    ys_all = ysrt[:, :, 0::2] + ysrt[:, :, 1::2]
    chunks = xs_all.reshape(NB, C, NCH, 128)
    reach = KREACH / alpha_enc.reshape(1, 3, 1)
    c_lo = np.ceil((chunks[:, :, :, 0] - reach - g0) / gd).astype(int)
    c_hi = np.floor((chunks[:, :, :, -1] + reach - g0) / gd).astype(int)
    ch_idx = np.arange(NCH)
    qfit = (c_lo[:, :, -1] + c_hi[:, :, -1] - c_lo[:, :, 0] - c_hi[:, :, 0]) / (
        2.0 * (NCH - 1)
    )
    q = int(round(float(np.median(qfit))))
    a = int((c_lo - q * ch_idx).min())
    whi = int((c_hi - q * ch_idx).max())
    wf = whi - a + 1
    off = min(a, 0)
    aoff = a - off
    mext = max(m, a + q * (NCH - 1) + wf) - off
    assert mext <= 512, f"psum extent {mext} > 512"
    assert wf <= 128, f"window {wf} too wide"

    # ---- packed device tensors ----
    o_bj = 0
    o_lbr = o_bj + njt
    o_low = o_lbr + 2 * NF
    o_lob = o_low + C * NS * 2 * C * NBASIS
    n128 = o_lob + NTT * W24
    o_gbn = 2 * RIN
    o_w1 = o_gbn + 1
    o_w2 = o_w1 + KW * ROUT
    o_w3 = o_w2 + KW * ROUT
    o_b123 = o_w3 + KW * ROUT
    o_linw = o_b123 + 3
    n32 = o_linw + 2 * NF
    o_xtr = 0
    o_eps = o_xtr + C * NTAR
    nb_ = o_eps + NF * NS
    o_gx = NBL * NCH * C

    p128 = np.zeros((128, n128), np.float32)
    gpad = np.zeros(njt * 128, np.float32)
    gpad[:m] = g.astype(np.float32)
    p128[:, o_bj : o_bj + njt] = (-alpha_int * gpad).reshape(njt, 128).T
    perm = np.array(
        [h * 15 + k * C + c for h in range(2) for c in range(C) for k in range(NBASIS)]
    )
    p128[:, o_lbr : o_lbr + 2 * NF] = linb[perm][None, :]
    lo = KAPPA * loW.reshape(NBASIS, C, 2 * C)  # (k, c, d)
    loBs_vec = (
        np.broadcast_to(lo.transpose(1, 2, 0)[:, None, :, :], (C, NS, 2 * C, NBASIS))
        .reshape(C * NS * 2 * C * NBASIS)
        .astype(np.float32)
    )
    p128[:, o_low : o_low + C * NS * 2 * C * NBASIS] = loBs_vec[None, :]
    p128[:, o_lob : o_lob + NTT * W24] = np.tile(lob, NTT * NS)[None, :]

    p32 = np.zeros((ROUT, n32), np.float32)
    p32[0:C, 0 : 2 * RIN] = np.concatenate([KAPPA * gW[0:3], gW[3:6]], axis=1)
    p32[0:RIN, o_gbn] = gb
    p32[0:RIN, o_w1 : o_w1 + KW * ROUT] = w1.transpose(1, 2, 0).reshape(RIN, -1)
    p32[0:ROUT, o_w2 : o_w2 + KW * ROUT] = w2.transpose(1, 2, 0).reshape(ROUT, -1)
    p32[0:ROUT, o_w3 : o_w3 + KW * ROUT] = w3.transpose(1, 2, 0).reshape(ROUT, -1)
    p32[0:ROUT, o_b123 : o_b123 + 3] = np.stack([b1, b2, b3], axis=1)
    p32[0:ROUT, o_linw : o_linw + 2 * NF] = linW[:, perm]

    ion = np.zeros((1, 2 + mext), np.float32)
    ion[0, 0] = EPS / KAPPA
    ion[0, 2:] = 1.0
    ion = ion.astype(ml_dtypes.bfloat16)

    xsr = xs_all.reshape(NB, C, NCH, 128).transpose(0, 3, 2, 1).reshape(NB, 128, -1)
    gxrow = (g0 + gd * (np.arange(mext) + off)).astype(np.float32)
    pball = np.empty((NB, 128, nb_), np.float32)
    pball[:, :, o_xtr : o_xtr + C * NTAR] = np.broadcast_to(
        x_out.transpose(0, 2, 1).reshape(NB, 1, C * NTAR), (NB, 128, C * NTAR)
    )
    pball[:, :, o_eps :] = np.broadcast_to(
        eps_noise.transpose(1, 2, 0).reshape(NB, 1, NF * NS), (NB, 128, NF * NS)
    )
    yi = np.empty((NB, 128, NCH * C * 2), np.float32)
    yi[:, :, 0::2] = 2.0
    yi[:, :, 1::2] = (
        ys_all.reshape(NB, C, NCH, 128).transpose(0, 3, 2, 1).reshape(NB, 128, -1)
    )
    yi = yi.astype(ml_dtypes.bfloat16)

    in_maps = []
    for core in range(NCORES):
        bsl = slice(core * NBL, (core + 1) * NBL)
        xgc = np.empty((128, NBL * NCH * C + mext), np.float32)
        for bl in range(NBL):
            xgc[:, bl * NCH * C : (bl + 1) * NCH * C] = xsr[core * NBL + bl]
        xgc[:, NBL * NCH * C :] = gxrow[None, :]
        in_maps.append(
            {
                "xg": xgc,
                "p128": p128,
                "p32": p32,
                "ion": ion,
                "pb": pball[bsl].copy(),
                "yi": np.ascontiguousarray(yi[bsl]),
            }
        )
    key = (m, q, aoff, wf, mext, -off, _build.alpha_int, tuple(_build.alpha_enc))
    return key, in_maps


def kernel(**inputs):
    key, in_maps = _prep(inputs)
    if key not in _CACHE:
        _CACHE[key] = _build(*key[:6])
    nc = _CACHE[key]
    res = bass_utils.run_bass_kernel_spmd(nc, in_maps, core_ids=list(range(NCORES)))
    outs = [res.results[c]["out"] for c in range(NCORES)]  # each (NS, NBL, NTAR, 6)
    full = np.concatenate(outs, axis=1)  # (NS, NB, NTAR, 6)
    return full.astype(np.float32)


# revision 31
# speedup vs baseline: 1.1046x; 1.0111x over previous
"""Trainium2 Bass kernel for a latent ConvCNP (gaussian encoder -> CNN ->
latent samples -> gaussian interpolator), data-parallel over batch on 8
NeuronCores.

v5: sorted + pair-merged windowed encoder (see v4 notes) with packed input
DMAs (3 constant packs + 2 per-batch packs instead of 21 transfers),
per-channel table pipelining, a single merged h_grid psum + add, a
manually placed natural_log_exp act-table load (3 table loads total), and
per-batch softplus + output DMAs so batch 0's results leave the core while
batch 1 is still in flight.

Contract: kernel(**inputs) takes the full unsharded inputs (numpy) and
returns the full (NS, nb, ntar, 2C) output.
"""

import sys

sys.path.insert(0, "/opt/trn_rl_repo")

import math

import ml_dtypes
import numpy as np

import concourse.bacc as bacc
import concourse.mybir as mybir
import concourse.tile as tile
from concourse import bass_utils
from concourse.ap import AP
from concourse.tile_rust import add_dep_helper

F32 = mybir.dt.float32
F32R = mybir.dt.float32r
BF16 = mybir.dt.bfloat16
AF = mybir.ActivationFunctionType
ALU = mybir.AluOpType

# problem constants (fixed by the reference problem)
EPS = 1e-6
C = 3
NBASIS = 5
NS = 4
RIN = 16
ROUT = 32
KW = 5
NB = 16          # full batch
NPTS = 2048
NTAR = 256
NCORES = 8
NBL = NB // NCORES   # batches per core
NPM = NPTS // 2      # pair-merged points
NCH = NPM // 128     # 8 chunks per (b, c)
KAPPA = math.sqrt(math.pi) / 2.0  # exp(-x^2) = KAPPA * Derivative_Erf(x)
KREACH = 4.0                      # window reach in units of 1/alpha
NF = C * NBASIS
NTT = NTAR // 128
W24 = NS * 2 * C

_CACHE = {}


def _build(m, q, aoff, wf, mext, moff):
    """Per-core Bass program. m = grid cols; chunk ch's window occupies psum
    cols [aoff + q*ch, +wf); grid col j lives at psum col j + moff (the gx
    input content is shifted to match)."""
    njt = (m + 127) // 128
    mts = [128] * (m // 128) + ([m % 128] if m % 128 else [])
    mp = m + 4  # padded conv width

    alpha_enc = _build.alpha_enc
    alpha_int = _build.alpha_int

    # packed-tensor column offsets (fp32 cols)
    # pack128: gx | bj | linbr | lowb | lobb
    o_bj = 0
    o_lbr = o_bj + njt
    o_low = o_lbr + 2 * NF
    o_lob = o_low + C * NS * 2 * C * NBASIS  # loBs: (c, s, d, k)
    n128 = o_lob + NTT * W24
    # pack32: gwab | gbn | w1 | w2 | w3 | b123 | linw
    o_gbn = 2 * RIN
    o_w1 = o_gbn + 1
    o_w2 = o_w1 + KW * ROUT
    o_w3 = o_w2 + KW * ROUT
    o_b123 = o_w3 + KW * ROUT
    o_linw = o_b123 + 3
    n32 = o_linw + 2 * NF
    # packb: xtr | epsb ; xg pack: xs (both b) | gx
    o_xtr = 0
    o_eps = o_xtr + C * NTAR
    nb_ = o_eps + NF * NS
    o_gx = NBL * NCH * C
    nxg = o_gx + mext

    nc = bacc.Bacc("TRN2", target_bir_lowering=False, debug=False)

    d_xg = nc.dram_tensor("xg", [128, nxg], F32, kind="ExternalInput")
    d_p128 = nc.dram_tensor("p128", [128, n128], F32, kind="ExternalInput")
    d_p32 = nc.dram_tensor("p32", [ROUT, n32], F32, kind="ExternalInput")
    d_ion = nc.dram_tensor("ion", [1, 2 + mext], BF16, kind="ExternalInput")
    d_pb = nc.dram_tensor("pb", [NBL, 128, nb_], F32, kind="ExternalInput")
    d_yi = nc.dram_tensor("yi", [NBL, 128, NCH * C * 2], BF16, kind="ExternalInput")
    d_out = nc.dram_tensor("out", [NS, NBL, NTAR, 2 * C], F32, kind="ExternalOutput")

    with tile.TileContext(nc) as tc:
        import contextlib

        est = contextlib.ExitStack()
        with est:
            p_cst = est.enter_context(tc.tile_pool(name="cst", bufs=1))
            p_io = est.enter_context(tc.tile_pool(name="io", bufs=1))
            p_tab = est.enter_context(tc.tile_pool(name="tab", bufs=3))
            p_ei = est.enter_context(tc.tile_pool(name="ei", bufs=2 * njt))
            p_h = est.enter_context(tc.tile_pool(name="h", bufs=2))
            p_sm = est.enter_context(tc.tile_pool(name="sm", bufs=2))
            p_z = est.enter_context(tc.tile_pool(name="z", bufs=2))
            p_zz2 = est.enter_context(tc.tile_pool(name="zz2", bufs=2))
            ps_e = est.enter_context(tc.tile_pool(name="pse", bufs=3, space="PSUM"))
            ps_r = est.enter_context(tc.tile_pool(name="psr", bufs=1, space="PSUM"))
            ps_c = est.enter_context(tc.tile_pool(name="psc", bufs=2, space="PSUM"))
            ps_h = est.enter_context(tc.tile_pool(name="psh", bufs=1, space="PSUM"))
            ps_o = est.enter_context(tc.tile_pool(name="pso", bufs=1, space="PSUM"))

            # ---- input DMAs: per-batch packs first (encoder head), then
            # constants ----
            pbs, yis = [], []
            ion = p_cst.tile([1, 2 + mext], BF16)
            nc.sync.dma_start(ion[:], d_ion.ap())
            xg = p_cst.tile([128, nxg], F32)
            nc.sync.dma_start(xg[:], d_xg.ap())
            for b in range(NBL):
                pb = p_io.tile([128, nb_], F32, tag=f"pb{b}")
                nc.sync.dma_start(pb[:], d_pb.ap()[b])
                pbs.append(pb)
            p128 = p_cst.tile([128, n128], F32)
            nc.sync.dma_start(p128[:], d_p128.ap())
            for b in range(NBL):
                yi = p_io.tile([128, NCH * C * 2], BF16, tag=f"yi{b}")
                nc.sync.dma_start(yi[:], d_yi.ap()[b])
                yis.append(yi)
            p32 = p_cst.tile([ROUT, n32], F32R)
            nc.sync.dma_start(p32[:], d_p32.ap().bitcast(F32R))

            bj = p128[:, o_bj : o_bj + njt]
            linbr = p128[:, o_lbr : o_lbr + 2 * NF]
            loBs = p128[:, o_low : o_low + C * NS * 2 * C * NBASIS]
            lobb = p128[:, o_lob : o_lob + NTT * W24]
            gwab = p32[0:C, 0 : 2 * RIN]
            gbn = p32[0:RIN, o_gbn : o_gbn + 1].bitcast(F32)
            w1 = p32[0:RIN, o_w1 : o_w1 + KW * ROUT]
            w2 = p32[0:ROUT, o_w2 : o_w2 + KW * ROUT]
            w3 = p32[0:ROUT, o_w3 : o_w3 + KW * ROUT]
            b123 = p32[0:ROUT, o_b123 : o_b123 + 3].bitcast(F32)
            linw = p32[0:ROUT, o_linw : o_linw + 2 * NF]

            # conv act tiles (dedicated; pads zeroed once on Pool) + merged ot
            h0cs = [
                p_cst.tile([RIN, mp], F32R, name=f"h0c{i}") for i in range(NBL)
            ]
            h1cs = [
                p_cst.tile([ROUT, mp], F32R, name=f"h1c{i}") for i in range(NBL)
            ]
            h2cs = [
                p_cst.tile([ROUT, mp], F32R, name=f"h2c{i}") for i in range(NBL)
            ]
            for t in h0cs + h1cs + h2cs:
                nc.gpsimd.memset(t[:, 0:2].bitcast(F32), 0.0)
                nc.gpsimd.memset(t[:, 2 + m : mp].bitcast(F32), 0.0)
            ot = p_cst.tile([128, NBL * NTT * W24], F32)  # (b, tt, s, d)

            # ---- encoder tables (DErf, ACT queue head) + accumulation ----
            # d6[p, (ch, r)] = gx[aoff + q*ch + r] - xs[p, (ch, c)]
            gap = xg[:]
            win = AP(
                gap.tensor, gap.offset + o_gx + aoff,
                [list(gap.ap[0]), [q, NCH], [1, wf]],
            )
            psums = [[None] * C for _ in range(NBL)]
            for b in range(NBL):
                for c in range(C):
                    d6 = p_tab.tile([128, NCH * wf], F32, tag="d6")
                    xv = (
                        xg[:, b * NCH * C : (b + 1) * NCH * C]
                        .rearrange("p (ch c) -> p ch c", ch=NCH, c=C)[:, :, c]
                        .unsqueeze(2)
                        .broadcast_to([128, NCH, wf])
                    )
                    d6v = d6[:].rearrange("p (ch r) -> p ch r", ch=NCH, r=wf)
                    nc.vector.tensor_tensor(d6v, win, xv, op=ALU.subtract)
                    e6 = p_tab.tile([128, NCH * wf], BF16, tag="e6")
                    nc.scalar.activation(
                        e6[:], d6[:], AF.Derivative_Erf, scale=float(alpha_enc[0])
                    )
                    ps2 = ps_e.tile([2, mext], F32, tag="pse")
                    nc.tensor.matmul(
                        ps2[:], ion[0:1, 0:2], ion[0:1, 2:],
                        start=True, stop=False, skip_group_check=True,
                    )
                    for ch in range(NCH):
                        s0 = aoff + q * ch
                        nc.tensor.matmul(
                            ps2[:, s0 : s0 + wf],
                            yis[b][:, (ch * C + c) * 2 : (ch * C + c) * 2 + 2],
                            e6[:, ch * wf : (ch + 1) * wf],
                            start=False, stop=(ch == NCH - 1),
                            skip_group_check=True,
                        )
                    psums[b][c] = ps2

            # ---- psum -> staging, relocation, bridge ----
            fH0s, nh3s = [], []
            for b in range(NBL):
                hcat = p_h.tile([2, C * m], F32R, tag="hcat")
                for c in range(C):
                    nc.vector.tensor_copy(
                        hcat[:, c * m : (c + 1) * m].bitcast(F32),
                        psums[b][c][:, moff : moff + m],
                    )
                fH0 = p_h.tile([C, m], F32R, tag="fH0")
                fH1 = p_h.tile([C, m], F32R, tag="fH1")
                nc.sync.dma_start(
                    fH0[:], hcat[0:1].rearrange("one (c m) -> one c m", c=C, m=m)
                )
                nc.sync.dma_start(
                    fH1[:], hcat[1:2].rearrange("one (c m) -> one c m", c=C, m=m)
                )
                rec3 = p_h.tile([C, m], F32, tag="rec3")
                nc.vector.reciprocal_approx_fast(rec3[:], fH0[:].bitcast(F32))
                nh3 = p_h.tile([C, m], F32R, tag="nh3")
                nc.vector.tensor_tensor(
                    nh3[:], fH1[:].bitcast(F32), rec3[:], op=ALU.mult
                )
                fH0s.append(fH0)
                nh3s.append(nh3)

            # ---- interp gaussian tables for b0 (DErf phase); b1's run
            # later in a second DErf window after the sigmoid phase ----
            eis = [[], []]
            for jt in range(njt):
                jts = mts[jt]
                ei = p_ei.tile([128, C * NTAR], F32, tag="ei")
                nc.scalar.activation(
                    ei[:jts],
                    pbs[0][:jts, o_xtr : o_xtr + C * NTAR],
                    AF.Derivative_Erf,
                    bias=bj[:jts, jt : jt + 1], scale=float(alpha_int),
                )
                eis[0].append(ei)

            # ---- phase B (sigmoid table), batch-interleaved by stage ----
            h3s = [None, None]
            for b in range(NBL):
                rp = ps_r.tile([RIN, m], F32, tag="rp")
                nc.tensor.matmul(rp[:], gwab[:, :RIN], fH0s[b][:],
                                 start=True, stop=False, skip_group_check=True)
                nc.tensor.matmul(rp[:], gwab[:, RIN:], nh3s[b][:],
                                 start=False, stop=True, skip_group_check=True)
                nc.scalar.activation(
                    h0cs[b][:, 2 : 2 + m], rp[:], AF.Sigmoid, bias=gbn, scale=1.0
                )
            hins = [h0cs[0], h0cs[1]]
            for li, (wt, cin) in enumerate([(w1, RIN), (w2, ROUT), (w3, ROUT)]):
                for b in range(NBL):
                    cps = ps_c.tile([ROUT, m], F32, tag="cps")
                    for dk in range(KW):
                        nc.tensor.matmul(
                            cps[:], wt[:cin, dk * ROUT : (dk + 1) * ROUT],
                            hins[b][:cin, dk : dk + m],
                            start=(dk == 0), stop=(dk == KW - 1),
                        )
                    if li == 0:
                        nc.scalar.activation(
                            h1cs[b][:, 2 : 2 + m], cps[:], AF.Relu,
                            bias=b123[:, 0:1], scale=1.0,
                        )
                        hins[b] = h1cs[b]
                    elif li == 1:
                        nc.scalar.activation(
                            h2cs[b][:, 2 : 2 + m], cps[:], AF.Relu,
                            bias=b123[:, 1:2], scale=1.0,
                        )
                        hins[b] = h2cs[b]
                    else:
                        h3 = p_h.tile([ROUT, m], F32R, tag="h3")
                        nc.vector.tensor_scalar_add(h3[:], cps[:], b123[:, 2:3])
                        h3s[b] = h3

            # h_grid -> z (mu/sigma in split halves, (jt, c, k) order)
            zs = [None, None]
            hsigs = []
            for b in range(NBL):
                hgps = ps_h.tile([128, njt * 2 * NF], F32, tag="hgps")
                for jt in range(njt):
                    jts = mts[jt]
                    j0 = jt * 128
                    nc.tensor.matmul(
                        hgps[:jts, jt * 2 * NF : (jt + 1) * 2 * NF],
                        h3s[b][:, j0 : j0 + jts], linw,
                        start=True, stop=True, skip_group_check=True,
                    )
                hgsb = p_sm.tile([128, 2 * njt * NF], F32, tag="hgsb")
                nc.vector.tensor_tensor(
                    hgsb[:].rearrange(
                        "p (h jt t) -> p h jt t", h=2, jt=njt, t=NF
                    ),
                    hgps[:].rearrange(
                        "p (jt h t) -> p h jt t", jt=njt, h=2, t=NF
                    ),
                    linbr.rearrange("p (h t) -> p h t", h=2, t=NF)
                    .unsqueeze(2)
                    .broadcast_to([128, 2, njt, NF]),
                    op=ALU.add,
                )
                hs = p_sm.tile([128, njt * NF], F32, tag="hs")
                hsig = nc.scalar.activation(
                    hs[:], hgsb[:, njt * NF :], AF.Sigmoid
                )
                hsigs.append(hsig)
                nc.vector.tensor_scalar(
                    hs[:], hs[:], 0.9, 0.1, op0=ALU.mult, op1=ALU.add
                )
                z = p_z.tile([128, njt * NF * NS], F32, tag="z")
                zv = z[:].rearrange(
                    "p (jt c s k) -> p jt c s k", jt=njt, c=C, s=NS, k=NBASIS
                )
                hsv = (
                    hs[:]
                    .rearrange("p (jt c k) -> p jt c k", jt=njt, c=C, k=NBASIS)
                    .unsqueeze(3)
                    .broadcast_to([128, njt, C, NS, NBASIS])
                )
                ev = (
                    pbs[b][:, o_eps : o_eps + NF * NS]
                    .rearrange("p (k c s) -> p c s k", k=NBASIS, c=C, s=NS)
                    .unsqueeze(1)
                    .broadcast_to([128, njt, C, NS, NBASIS])
                )
                nc.vector.tensor_tensor(zv, hsv, ev, op=ALU.mult)
                muv = (
                    hgsb[:, : njt * NF]
                    .rearrange("p (jt c k) -> p jt c k", jt=njt, c=C, k=NBASIS)
                    .unsqueeze(3)
                    .broadcast_to([128, njt, C, NS, NBASIS])
                )
                nc.vector.tensor_tensor(zv, zv, muv, op=ALU.add)
                zs[b] = z

            # b1 interp tables (second DErf window, after all sigmoid ops)
            last_ei = None
            for jt in range(njt):
                jts = mts[jt]
                ei = p_ei.tile([128, C * NTAR], F32, tag="ei")
                ai = nc.scalar.activation(
                    ei[:jts],
                    pbs[1][:jts, o_xtr : o_xtr + C * NTAR],
                    AF.Derivative_Erf,
                    bias=bj[:jts, jt : jt + 1], scale=float(alpha_int),
                )
                for hsig in hsigs:
                    add_dep_helper(ai.ins, hsig.ins, sync=False)
                last_ei = ai
                eis[1].append(ei)

            # interp matmuls + per-batch softplus + out (one id6 table load
            # covers Abs/Relu/Exp/Ln for both batches)
            ld = mybir.InstLoadActFuncSet(
                name=nc.get_next_instruction_name(), ins=[], outs=[],
                act_func_set_id=6,
            )
            nc.scalar.add_instruction(ld)
            add_dep_helper(ld, last_ei.ins, sync=False)
            nsk = NS * NBASIS
            for b in range(NBL):
                for tt in range(NTT):
                    # P[t, (c, s, k)] = sum_j ei_c[j, t] * z[j, (c, s, k)]
                    # P[t, (c, s, k)] accumulated over grid tiles
                    po = ps_o.tile([128, C * nsk], F32, tag="po")
                    for c in range(C):
                        t0 = c * NTAR + tt * 128
                        for jt in range(njt):
                            jts = mts[jt]
                            nc.tensor.matmul(
                                po[:, c * nsk : (c + 1) * nsk],
                                eis[b][jt][:jts, t0 : t0 + 128],
                                zs[b][
                                    :jts,
                                    jt * C * nsk + c * nsk : jt * C * nsk
                                    + (c + 1) * nsk,
                                ],
                                start=(jt == 0), stop=(jt == njt - 1),
                                skip_group_check=True,
                            )
                    # zz1[(c,s,d)] = sum_k P[(c,s,k)] * loBs[(c,s,d,k)]
                    zzt = p_sm.tile([128, C * NS * 2 * C * NBASIS], F32, tag="zzt")
                    zztv = zzt[:].rearrange(
                        "p (cs d k) -> p cs d k", cs=C * NS, d=2 * C, k=NBASIS
                    )
                    pv = (
                        po[:]
                        .rearrange("p (cs k) -> p cs k", cs=C * NS, k=NBASIS)
                        .unsqueeze(2)
                        .broadcast_to([128, C * NS, 2 * C, NBASIS])
                    )
                    lov = loBs.rearrange(
                        "p (cs d k) -> p cs d k", cs=C * NS, d=2 * C, k=NBASIS
                    )
                    nc.vector.tensor_tensor(zztv, pv, lov, op=ALU.mult)
                    zz1 = p_sm.tile([128, C * NS * 2 * C], F32, tag="zz1")
                    nc.vector.reduce_sum(
                        zz1[:],
                        zzt[:].rearrange(
                            "p (csd k) -> p csd k", csd=C * NS * 2 * C, k=NBASIS
                        ),
                        axis=mybir.AxisListType.X,
                    )
                    osl = ot[:, (b * NTT + tt) * W24 : (b * NTT + tt + 1) * W24]
                    nc.vector.reduce_sum(
                        osl.rearrange("p (sd) -> p sd", sd=W24),
                        zz1[:].rearrange("p (c sd) -> p sd c", c=C, sd=W24),
                        axis=mybir.AxisListType.X,
                    )
                    nc.vector.tensor_tensor(
                        osl, osl, lobb[:, tt * W24 : (tt + 1) * W24], op=ALU.add
                    )
                # softplus on this batch's std cols
                ng = NTT * NS
                sv = ot[:, b * NTT * W24 : (b + 1) * NTT * W24].rearrange(
                    "p (g d) -> p g d", g=ng, d=2 * C
                )[:, :, C:]
                av = p_sm.tile([128, ng * C], F32, tag="av")
                avv = av[:].rearrange("p (g d) -> p g d", g=ng, d=C)
                a1 = nc.scalar.activation(avv, sv, AF.Abs)
                add_dep_helper(a1.ins, ld, sync=False)
                rv = p_sm.tile([128, ng * C], F32, tag="rv")
                rvv = rv[:].rearrange("p (g d) -> p g d", g=ng, d=C)
                a2 = nc.scalar.activation(rvv, sv, AF.Relu)
                add_dep_helper(a2.ins, ld, sync=False)
                ew = p_sm.tile([128, ng * C], F32, tag="ew")
                a3 = nc.scalar.activation(ew[:], av[:], AF.Exp, scale=-1.0)
                add_dep_helper(a3.ins, ld, sync=False)
                lw_ = p_sm.tile([128, ng * C], F32, tag="lw_")
                a4 = nc.scalar.activation(lw_[:], ew[:], AF.Ln, bias=1.0)
                add_dep_helper(a4.ins, ld, sync=False)
                lvv = lw_[:].rearrange("p (g d) -> p g d", g=ng, d=C)
                nc.vector.tensor_tensor(sv, rvv, lvv, op=ALU.add)
                for tt in range(NTT):
                    nc.sync.dma_start(
                        d_out.ap()[:, b, tt * 128 : (tt + 1) * 128, :].rearrange(
                            "s p d -> p s d"
                        ),
                        ot[
                            :, (b * NTT + tt) * W24 : (b * NTT + tt + 1) * W24
                        ].rearrange("p (s d) -> p s d", s=NS, d=2 * C),
                    )

    nc.compile()
    return nc


def _prep(inputs):
    """Host-side sorting/pair-merging/packing. Returns (key, per-core in_maps)."""
    x = np.ascontiguousarray(inputs["x"], dtype=np.float32)
    y = np.ascontiguousarray(inputs["y"], dtype=np.float32)
    x_out = np.ascontiguousarray(inputs["x_out"], dtype=np.float32)
    x_grid = np.asarray(inputs["x_grid"], dtype=np.float32)
    eps_noise = np.asarray(inputs["eps_noise"], dtype=np.float32)
    enc_sigma = np.asarray(inputs["enc_sigma"], dtype=np.float64)
    int_sigma = np.asarray(inputs["int_sigma"], dtype=np.float64)
    gW = np.asarray(inputs["gW"], dtype=np.float32)
    gb = np.asarray(inputs["gb"], dtype=np.float32)
    w1 = np.asarray(inputs["w1"], dtype=np.float32)
    b1 = np.asarray(inputs["b1"], dtype=np.float32)
    w2 = np.asarray(inputs["w2"], dtype=np.float32)
    b2 = np.asarray(inputs["b2"], dtype=np.float32)
    w3 = np.asarray(inputs["w3"], dtype=np.float32)
    b3 = np.asarray(inputs["b3"], dtype=np.float32)
    linW = np.asarray(inputs["linW"], dtype=np.float32)
    linb = np.asarray(inputs["linb"], dtype=np.float32)
    loW = np.asarray(inputs["loW"], dtype=np.float32)
    lob = np.asarray(inputs["lob"], dtype=np.float32)

    nb, npts, _ = x.shape
    assert nb == NB and npts == NPTS
    m = x_grid.shape[1]
    njt = (m + 127) // 128
    g = x_grid[0, :, 0].astype(np.float64)
    g0 = float(g[0])
    gd = float((g[-1] - g[0]) / (m - 1))

    s_enc = np.exp(enc_sigma) + EPS
    alpha_enc = 1.0 / (np.sqrt(2.0) * s_enc)
    assert np.ptp(alpha_enc) < 1e-9 * abs(alpha_enc[0]), "enc_sigma must be uniform"
    s_int = np.exp(int_sigma) + EPS
    assert np.ptp(s_int) < 1e-12 * abs(s_int.flat[0]), "int_sigma must be uniform"
    alpha_int = float(1.0 / (np.sqrt(2.0) * s_int.flat[0]))
    _build.alpha_enc = [float(a) for a in alpha_enc]
    _build.alpha_int = alpha_int

    # ---- sort + pair-merge points per (b, c); global affine window lattice
    srt = np.sort(x.transpose(0, 2, 1), axis=2)
    idx = np.argsort(x.transpose(0, 2, 1), axis=2, kind="stable")
    ysrt = np.take_along_axis(y.transpose(0, 2, 1), idx, axis=2)
    xs_all = 0.5 * (srt[:, :, 0::2] + srt[:, :, 1::2])
    ys_all = ysrt[:, :, 0::2] + ysrt[:, :, 1::2]
    chunks = xs_all.reshape(NB, C, NCH, 128)
    reach = KREACH / alpha_enc.reshape(1, 3, 1)
    c_lo = np.ceil((chunks[:, :, :, 0] - reach - g0) / gd).astype(int)
    c_hi = np.floor((chunks[:, :, :, -1] + reach - g0) / gd).astype(int)
    ch_idx = np.arange(NCH)
    qfit = (c_lo[:, :, -1] + c_hi[:, :, -1] - c_lo[:, :, 0] - c_hi[:, :, 0]) / (
        2.0 * (NCH - 1)
    )
    q = int(round(float(np.median(qfit))))
    a = int((c_lo - q * ch_idx).min())
    whi = int((c_hi - q * ch_idx).max())
    wf = whi - a + 1
    off = min(a, 0)
    aoff = a - off
    mext = max(m, a + q * (NCH - 1) + wf) - off
    assert mext <= 512, f"psum extent {mext} > 512"
    assert wf <= 128, f"window {wf} too wide"

    # ---- packed device tensors ----
    o_bj = 0
    o_lbr = o_bj + njt
    o_low = o_lbr + 2 * NF
    o_lob = o_low + C * NS * 2 * C * NBASIS
    n128 = o_lob + NTT * W24
    o_gbn = 2 * RIN
    o_w1 = o_gbn + 1
    o_w2 = o_w1 + KW * ROUT
    o_w3 = o_w2 + KW * ROUT
    o_b123 = o_w3 + KW * ROUT
    o_linw = o_b123 + 3
    n32 = o_linw + 2 * NF
    o_xtr = 0
    o_eps = o_xtr + C * NTAR
    nb_ = o_eps + NF * NS
    o_gx = NBL * NCH * C

    p128 = np.zeros((128, n128), np.float32)
    gpad = np.zeros(njt * 128, np.float32)
    gpad[:m] = g.astype(np.float32)
    p128[:, o_bj : o_bj + njt] = (-alpha_int * gpad).reshape(njt, 128).T
    perm = np.array(
        [h * 15 + k * C + c for h in range(2) for c in range(C) for k in range(NBASIS)]
    )
    p128[:, o_lbr : o_lbr + 2 * NF] = linb[perm][None, :]
    lo = KAPPA * loW.reshape(NBASIS, C, 2 * C)  # (k, c, d)
    loBs_vec = (
        np.broadcast_to(lo.transpose(1, 2, 0)[:, None, :, :], (C, NS, 2 * C, NBASIS))
        .reshape(C * NS * 2 * C * NBASIS)
        .astype(np.float32)
    )
    p128[:, o_low : o_low + C * NS * 2 * C * NBASIS] = loBs_vec[None, :]
    p128[:, o_lob : o_lob + NTT * W24] = np.tile(lob, NTT * NS)[None, :]

    p32 = np.zeros((ROUT, n32), np.float32)
    p32[0:C, 0 : 2 * RIN] = np.concatenate([KAPPA * gW[0:3], gW[3:6]], axis=1)
    p32[0:RIN, o_gbn] = gb
    p32[0:RIN, o_w1 : o_w1 + KW * ROUT] = w1.transpose(1, 2, 0).reshape(RIN, -1)
    p32[0:ROUT, o_w2 : o_w2 + KW * ROUT] = w2.transpose(1, 2, 0).reshape(ROUT, -1)
    p32[0:ROUT, o_w3 : o_w3 + KW * ROUT] = w3.transpose(1, 2, 0).reshape(ROUT, -1)
    p32[0:ROUT, o_b123 : o_b123 + 3] = np.stack([b1, b2, b3], axis=1)
    p32[0:ROUT, o_linw : o_linw + 2 * NF] = linW[:, perm]

    ion = np.zeros((1, 2 + mext), np.float32)
    ion[0, 0] = EPS / KAPPA
    ion[0, 2:] = 1.0
    ion = ion.astype(ml_dtypes.bfloat16)

    xsr = xs_all.reshape(NB, C, NCH, 128).transpose(0, 3, 2, 1).reshape(NB, 128, -1)
    gxrow = (g0 + gd * (np.arange(mext) + off)).astype(np.float32)
    pball = np.empty((NB, 128, nb_), np.float32)
    pball[:, :, o_xtr : o_xtr + C * NTAR] = np.broadcast_to(
        x_out.transpose(0, 2, 1).reshape(NB, 1, C * NTAR), (NB, 128, C * NTAR)
    )
    pball[:, :, o_eps :] = np.broadcast_to(
        eps_noise.transpose(1, 2, 0).reshape(NB, 1, NF * NS), (NB, 128, NF * NS)
    )
    yi = np.empty((NB, 128, NCH * C * 2), np.float32)
    yi[:, :, 0::2] = 2.0
    yi[:, :, 1::2] = (
        ys_all.reshape(NB, C, NCH, 128).transpose(0, 3, 2, 1).reshape(NB, 128, -1)
    )
    yi = yi.astype(ml_dtypes.bfloat16)

    in_maps = []
    for core in range(NCORES):
        bsl = slice(core * NBL, (core + 1) * NBL)
        xgc = np.empty((128, NBL * NCH * C + mext), np.float32)
        for bl in range(NBL):
            xgc[:, bl * NCH * C : (bl + 1) * NCH * C] = xsr[core * NBL + bl]
        xgc[:, NBL * NCH * C :] = gxrow[None, :]
        in_maps.append(
            {
                "xg": xgc,
                "p128": p128,
                "p32": p32,
                "ion": ion,
                "pb": pball[bsl].copy(),
                "yi": np.ascontiguousarray(yi[bsl]),
            }
        )
    key = (m, q, aoff, wf, mext, -off, _build.alpha_int, tuple(_build.alpha_enc))
    return key, in_maps


def kernel(**inputs):
    key, in_maps = _prep(inputs)
    if key not in _CACHE:
        _CACHE[key] = _build(*key[:6])
    nc = _CACHE[key]
    res = bass_utils.run_bass_kernel_spmd(nc, in_maps, core_ids=list(range(NCORES)))
    outs = [res.results[c]["out"] for c in range(NCORES)]  # each (NS, NBL, NTAR, 6)
    full = np.concatenate(outs, axis=1)  # (NS, NB, NTAR, 6)
    return full.astype(np.float32)


# revision 32
# speedup vs baseline: 1.1122x; 1.0069x over previous
"""Trainium2 Bass kernel for a latent ConvCNP (gaussian encoder -> CNN ->
latent samples -> gaussian interpolator), data-parallel over batch on 8
NeuronCores.

v5: sorted + pair-merged windowed encoder (see v4 notes) with packed input
DMAs (3 constant packs + 2 per-batch packs instead of 21 transfers),
per-channel table pipelining, a single merged h_grid psum + add, a
manually placed natural_log_exp act-table load (3 table loads total), and
per-batch softplus + output DMAs so batch 0's results leave the core while
batch 1 is still in flight.

Contract: kernel(**inputs) takes the full unsharded inputs (numpy) and
returns the full (NS, nb, ntar, 2C) output.
"""

import sys

sys.path.insert(0, "/opt/trn_rl_repo")

import math

import ml_dtypes
import numpy as np

import concourse.bacc as bacc
import concourse.mybir as mybir
import concourse.tile as tile
from concourse import bass_utils
from concourse.ap import AP
from concourse.tile_rust import add_dep_helper

F32 = mybir.dt.float32
F32R = mybir.dt.float32r
BF16 = mybir.dt.bfloat16
AF = mybir.ActivationFunctionType
ALU = mybir.AluOpType

# problem constants (fixed by the reference problem)
EPS = 1e-6
C = 3
NBASIS = 5
NS = 4
RIN = 16
ROUT = 32
KW = 5
NB = 16          # full batch
NPTS = 2048
NTAR = 256
NCORES = 8
NBL = NB // NCORES   # batches per core
NPM = NPTS // 2      # pair-merged points
NCH = NPM // 128     # 8 chunks per (b, c)
KAPPA = math.sqrt(math.pi) / 2.0  # exp(-x^2) = KAPPA * Derivative_Erf(x)
KREACH = 4.0                      # window reach in units of 1/alpha
NF = C * NBASIS
NTT = NTAR // 128
W24 = NS * 2 * C

_CACHE = {}


def _build(m, q, aoff, wf, mext, moff):
    """Per-core Bass program. m = grid cols; chunk ch's window occupies psum
    cols [aoff + q*ch, +wf); grid col j lives at psum col j + moff (the gx
    input content is shifted to match)."""
    njt = (m + 127) // 128
    mts = [128] * (m // 128) + ([m % 128] if m % 128 else [])
    mp = m + 4  # padded conv width

    alpha_enc = _build.alpha_enc
    alpha_int = _build.alpha_int

    # packed-tensor column offsets (fp32 cols)
    # pack128: gx | bj | linbr | lowb | lobb
    o_bj = 0
    o_lbr = o_bj + njt
    o_low = o_lbr + 2 * NF
    o_lob = o_low + C * NS * 2 * C * NBASIS  # loBs: (c, s, d, k)
    n128 = o_lob + NTT * W24
    # pack32: gwab | gbn | w1 | w2 | w3 | b123 | linw
    o_gbn = 2 * RIN
    o_w1 = o_gbn + 1
    o_w2 = o_w1 + KW * ROUT
    o_w3 = o_w2 + KW * ROUT
    o_b123 = o_w3 + KW * ROUT
    o_linw = o_b123 + 3
    n32 = o_linw + 2 * NF
    # packb: xtr | epsb ; xg pack: xs (both b) | gx
    o_xtr = 0
    o_eps = o_xtr + C * NTAR
    nb_ = o_eps + NF * NS
    o_gx = NBL * NCH * C
    nxg = o_gx + mext

    nc = bacc.Bacc("TRN2", target_bir_lowering=False, debug=False)

    d_xg = nc.dram_tensor("xg", [128, nxg], F32, kind="ExternalInput")
    d_p128 = nc.dram_tensor("p128", [128, n128], F32, kind="ExternalInput")
    d_p32 = nc.dram_tensor("p32", [ROUT, n32], F32, kind="ExternalInput")
    d_ion = nc.dram_tensor("ion", [1, 2 + mext], BF16, kind="ExternalInput")
    d_pb = nc.dram_tensor("pb", [NBL, 128, nb_], F32, kind="ExternalInput")
    d_yi = nc.dram_tensor("yi", [NBL, 128, NCH * C * 2], BF16, kind="ExternalInput")
    d_out = nc.dram_tensor("out", [NS, NBL, NTAR, 2 * C], F32, kind="ExternalOutput")

    with tile.TileContext(nc) as tc:
        import contextlib

        est = contextlib.ExitStack()
        with est:
            p_cst = est.enter_context(tc.tile_pool(name="cst", bufs=1))
            p_io = est.enter_context(tc.tile_pool(name="io", bufs=1))
            p_tab = est.enter_context(tc.tile_pool(name="tab", bufs=3))
            p_ei = est.enter_context(tc.tile_pool(name="ei", bufs=2 * njt))
            p_h = est.enter_context(tc.tile_pool(name="h", bufs=2))
            p_sm = est.enter_context(tc.tile_pool(name="sm", bufs=2))
            p_z = est.enter_context(tc.tile_pool(name="z", bufs=2))
            p_zz2 = est.enter_context(tc.tile_pool(name="zz2", bufs=2))
            ps_e = est.enter_context(tc.tile_pool(name="pse", bufs=3, space="PSUM"))
            ps_r = est.enter_context(tc.tile_pool(name="psr", bufs=1, space="PSUM"))
            ps_c = est.enter_context(tc.tile_pool(name="psc", bufs=2, space="PSUM"))
            ps_h = est.enter_context(tc.tile_pool(name="psh", bufs=1, space="PSUM"))
            ps_o = est.enter_context(tc.tile_pool(name="pso", bufs=1, space="PSUM"))

            # ---- input DMAs: per-batch packs first (encoder head), then
            # constants ----
            pbs, yis = [], []
            ion = p_cst.tile([1, 2 + mext], BF16)
            nc.sync.dma_start(ion[:], d_ion.ap())
            xg = p_cst.tile([128, nxg], F32)
            nc.sync.dma_start(xg[:], d_xg.ap())
            for b in range(NBL):
                pb = p_io.tile([128, nb_], F32, tag=f"pb{b}")
                nc.sync.dma_start(pb[:], d_pb.ap()[b])
                pbs.append(pb)
            p128 = p_cst.tile([128, n128], F32)
            nc.sync.dma_start(p128[:], d_p128.ap())
            for b in range(NBL):
                yi = p_io.tile([128, NCH * C * 2], BF16, tag=f"yi{b}")
                nc.sync.dma_start(yi[:], d_yi.ap()[b])
                yis.append(yi)
            p32 = p_cst.tile([ROUT, n32], F32R)
            nc.sync.dma_start(p32[:], d_p32.ap().bitcast(F32R))

            bj = p128[:, o_bj : o_bj + njt]
            linbr = p128[:, o_lbr : o_lbr + 2 * NF]
            loBs = p128[:, o_low : o_low + C * NS * 2 * C * NBASIS]
            lobb = p128[:, o_lob : o_lob + NTT * W24]
            gwab = p32[0:C, 0 : 2 * RIN]
            gbn = p32[0:RIN, o_gbn : o_gbn + 1].bitcast(F32)
            w1 = p32[0:RIN, o_w1 : o_w1 + KW * ROUT]
            w2 = p32[0:ROUT, o_w2 : o_w2 + KW * ROUT]
            w3 = p32[0:ROUT, o_w3 : o_w3 + KW * ROUT]
            b123 = p32[0:ROUT, o_b123 : o_b123 + 3].bitcast(F32)
            linw = p32[0:ROUT, o_linw : o_linw + 2 * NF]

            # conv act tiles (dedicated; pads zeroed once on Pool) + merged ot
            h0cs = [
                p_cst.tile([RIN, mp], F32R, name=f"h0c{i}") for i in range(NBL)
            ]
            h1cs = [
                p_cst.tile([ROUT, mp], F32R, name=f"h1c{i}") for i in range(NBL)
            ]
            h2cs = [
                p_cst.tile([ROUT, mp], F32R, name=f"h2c{i}") for i in range(NBL)
            ]
            for t in h0cs + h1cs + h2cs:
                nc.gpsimd.memset(t[:, 0:2].bitcast(F32), 0.0)
                nc.gpsimd.memset(t[:, 2 + m : mp].bitcast(F32), 0.0)
            ot = p_cst.tile([128, NBL * NTT * W24], F32)  # (b, tt, s, d)

            # ---- encoder tables (DErf, ACT queue head) + accumulation ----
            # d6[p, (ch, r)] = gx[aoff + q*ch + r] - xs[p, (ch, c)]
            gap = xg[:]
            win = AP(
                gap.tensor, gap.offset + o_gx + aoff,
                [list(gap.ap[0]), [q, NCH], [1, wf]],
            )
            psums = [[None] * C for _ in range(NBL)]
            for b in range(NBL):
                for c in range(C):
                    d6 = p_tab.tile([128, NCH * wf], F32, tag="d6")
                    xv = (
                        xg[:, b * NCH * C : (b + 1) * NCH * C]
                        .rearrange("p (ch c) -> p ch c", ch=NCH, c=C)[:, :, c]
                        .unsqueeze(2)
                        .broadcast_to([128, NCH, wf])
                    )
                    d6v = d6[:].rearrange("p (ch r) -> p ch r", ch=NCH, r=wf)
                    nc.vector.tensor_tensor(d6v, win, xv, op=ALU.subtract)
                    e6 = p_tab.tile([128, NCH * wf], BF16, tag="e6")
                    nc.scalar.activation(
                        e6[:], d6[:], AF.Derivative_Erf, scale=float(alpha_enc[0])
                    )
                    ps2 = ps_e.tile([2, mext], F32, tag="pse")
                    nc.tensor.matmul(
                        ps2[:], ion[0:1, 0:2], ion[0:1, 2:],
                        start=True, stop=False, skip_group_check=True,
                    )
                    for ch in range(NCH):
                        s0 = aoff + q * ch
                        nc.tensor.matmul(
                            ps2[:, s0 : s0 + wf],
                            yis[b][:, (ch * C + c) * 2 : (ch * C + c) * 2 + 2],
                            e6[:, ch * wf : (ch + 1) * wf],
                            start=False, stop=(ch == NCH - 1),
                            skip_group_check=True,
                        )
                    psums[b][c] = ps2

            # ---- psum -> staging, relocation, bridge ----
            fH0s, nh3s = [], []
            for b in range(NBL):
                hcat = p_h.tile([2, C * m], F32R, tag="hcat")
                for c in range(C):
                    nc.vector.tensor_copy(
                        hcat[:, c * m : (c + 1) * m].bitcast(F32),
                        psums[b][c][:, moff : moff + m],
                    )
                fH0 = p_h.tile([C, m], F32R, tag="fH0")
                fH1 = p_h.tile([C, m], F32R, tag="fH1")
                nc.sync.dma_start(
                    fH0[:], hcat[0:1].rearrange("one (c m) -> one c m", c=C, m=m)
                )
                nc.sync.dma_start(
                    fH1[:], hcat[1:2].rearrange("one (c m) -> one c m", c=C, m=m)
                )
                rec3 = p_h.tile([C, m], F32, tag="rec3")
                nc.vector.reciprocal_approx_fast(rec3[:], fH0[:].bitcast(F32))
                nh3 = p_h.tile([C, m], F32R, tag="nh3")
                nc.vector.tensor_tensor(
                    nh3[:], fH1[:].bitcast(F32), rec3[:], op=ALU.mult
                )
                fH0s.append(fH0)
                nh3s.append(nh3)

            # ---- interp gaussian tables for b0 (DErf phase); b1's run
            # later in a second DErf window after the sigmoid phase ----
            eis = [[], []]
            for jt in range(njt):
                jts = mts[jt]
                ei = p_ei.tile([128, C * NTAR], F32, tag="ei")
                nc.scalar.activation(
                    ei[:jts],
                    pbs[0][:jts, o_xtr : o_xtr + C * NTAR],
                    AF.Derivative_Erf,
                    bias=bj[:jts, jt : jt + 1], scale=float(alpha_int),
                )
                eis[0].append(ei)

            # ---- phase B (sigmoid table), batch-interleaved by stage ----
            h3s = [None, None]
            rsigs = []
            for b in range(NBL):
                rp = ps_r.tile([RIN, m], F32, tag="rp")
                nc.tensor.matmul(rp[:], gwab[:, :RIN], fH0s[b][:],
                                 start=True, stop=False, skip_group_check=True)
                nc.tensor.matmul(rp[:], gwab[:, RIN:], nh3s[b][:],
                                 start=False, stop=True, skip_group_check=True)
                rsig = nc.scalar.activation(
                    h0cs[b][:, 2 : 2 + m], rp[:], AF.Sigmoid, bias=gbn, scale=1.0
                )
                rsigs.append(rsig)
            hins = [h0cs[0], h0cs[1]]
            for li, (wt, cin) in enumerate([(w1, RIN), (w2, ROUT), (w3, ROUT)]):
                for b in range(NBL):
                    cps = ps_c.tile([ROUT, m], F32, tag="cps")
                    for dk in range(KW):
                        nc.tensor.matmul(
                            cps[:], wt[:cin, dk * ROUT : (dk + 1) * ROUT],
                            hins[b][:cin, dk : dk + m],
                            start=(dk == 0), stop=(dk == KW - 1),
                        )
                    if li == 0:
                        nc.vector.tensor_scalar(
                            h1cs[b][:, 2 : 2 + m], cps[:], b123[:, 0:1], 0.0,
                            op0=ALU.add, op1=ALU.max,
                        )
                        hins[b] = h1cs[b]
                    elif li == 1:
                        nc.vector.tensor_scalar(
                            h2cs[b][:, 2 : 2 + m], cps[:], b123[:, 1:2], 0.0,
                            op0=ALU.add, op1=ALU.max,
                        )
                        hins[b] = h2cs[b]
                    else:
                        h3 = p_h.tile([ROUT, m], F32R, tag="h3")
                        nc.vector.tensor_scalar_add(h3[:], cps[:], b123[:, 2:3])
                        h3s[b] = h3

            # h_grid -> z (mu/sigma in split halves, (jt, c, k) order)
            zs = [None, None]
            hsigs = []
            for b in range(NBL):
                hgps = ps_h.tile([128, njt * 2 * NF], F32, tag="hgps")
                for jt in range(njt):
                    jts = mts[jt]
                    j0 = jt * 128
                    nc.tensor.matmul(
                        hgps[:jts, jt * 2 * NF : (jt + 1) * 2 * NF],
                        h3s[b][:, j0 : j0 + jts], linw,
                        start=True, stop=True, skip_group_check=True,
                    )
                hgsb = p_sm.tile([128, 2 * njt * NF], F32, tag="hgsb")
                nc.vector.tensor_tensor(
                    hgsb[:].rearrange(
                        "p (h jt t) -> p h jt t", h=2, jt=njt, t=NF
                    ),
                    hgps[:].rearrange(
                        "p (jt h t) -> p h jt t", jt=njt, h=2, t=NF
                    ),
                    linbr.rearrange("p (h t) -> p h t", h=2, t=NF)
                    .unsqueeze(2)
                    .broadcast_to([128, 2, njt, NF]),
                    op=ALU.add,
                )
                hs = p_sm.tile([128, njt * NF], F32, tag="hs")
                hsig = nc.scalar.activation(
                    hs[:], hgsb[:, njt * NF :], AF.Exp, scale=-1.0
                )
                hsigs.append(hsig)
                nc.vector.tensor_scalar_add(hs[:], hs[:], 1.0)
                hrec = p_sm.tile([128, njt * NF], F32, tag="hrec")
                nc.vector.reciprocal_approx_fast(hrec[:], hs[:])
                nc.vector.tensor_scalar(
                    hs[:], hrec[:], 0.9, 0.1, op0=ALU.mult, op1=ALU.add
                )
                z = p_z.tile([128, njt * NF * NS], F32, tag="z")
                zv = z[:].rearrange(
                    "p (jt c s k) -> p jt c s k", jt=njt, c=C, s=NS, k=NBASIS
                )
                hsv = (
                    hs[:]
                    .rearrange("p (jt c k) -> p jt c k", jt=njt, c=C, k=NBASIS)
                    .unsqueeze(3)
                    .broadcast_to([128, njt, C, NS, NBASIS])
                )
                ev = (
                    pbs[b][:, o_eps : o_eps + NF * NS]
                    .rearrange("p (k c s) -> p c s k", k=NBASIS, c=C, s=NS)
                    .unsqueeze(1)
                    .broadcast_to([128, njt, C, NS, NBASIS])
                )
                nc.vector.tensor_tensor(zv, hsv, ev, op=ALU.mult)
                muv = (
                    hgsb[:, : njt * NF]
                    .rearrange("p (jt c k) -> p jt c k", jt=njt, c=C, k=NBASIS)
                    .unsqueeze(3)
                    .broadcast_to([128, njt, C, NS, NBASIS])
                )
                nc.vector.tensor_tensor(zv, zv, muv, op=ALU.add)
                zs[b] = z

            # b1 interp tables (second DErf window, after all sigmoid ops)
            last_ei = None
            for jt in range(njt):
                jts = mts[jt]
                ei = p_ei.tile([128, C * NTAR], F32, tag="ei")
                ai = nc.scalar.activation(
                    ei[:jts],
                    pbs[1][:jts, o_xtr : o_xtr + C * NTAR],
                    AF.Derivative_Erf,
                    bias=bj[:jts, jt : jt + 1], scale=float(alpha_int),
                )
                for rsig in rsigs:
                    add_dep_helper(ai.ins, rsig.ins, sync=False)
                last_ei = ai
                eis[1].append(ei)
            for hsig in hsigs:
                add_dep_helper(hsig.ins, last_ei.ins, sync=False)

            # interp matmuls + per-batch softplus + out (one id6 table load
            # covers Abs/Relu/Exp/Ln for both batches)
            ld = mybir.InstLoadActFuncSet(
                name=nc.get_next_instruction_name(), ins=[], outs=[],
                act_func_set_id=6,
            )
            nc.scalar.add_instruction(ld)
            add_dep_helper(ld, last_ei.ins, sync=False)
            nsk = NS * NBASIS
            for b in range(NBL):
                for tt in range(NTT):
                    # P[t, (c, s, k)] = sum_j ei_c[j, t] * z[j, (c, s, k)]
                    # P[t, (c, s, k)] accumulated over grid tiles
                    po = ps_o.tile([128, C * nsk], F32, tag="po")
                    for c in range(C):
                        t0 = c * NTAR + tt * 128
                        for jt in range(njt):
                            jts = mts[jt]
                            nc.tensor.matmul(
                                po[:, c * nsk : (c + 1) * nsk],
                                eis[b][jt][:jts, t0 : t0 + 128],
                                zs[b][
                                    :jts,
                                    jt * C * nsk + c * nsk : jt * C * nsk
                                    + (c + 1) * nsk,
                                ],
                                start=(jt == 0), stop=(jt == njt - 1),
                                skip_group_check=True,
                            )
                    # zz1[(c,s,d)] = sum_k P[(c,s,k)] * loBs[(c,s,d,k)]
                    zzt = p_sm.tile([128, C * NS * 2 * C * NBASIS], F32, tag="zzt")
                    zztv = zzt[:].rearrange(
                        "p (cs d k) -> p cs d k", cs=C * NS, d=2 * C, k=NBASIS
                    )
                    pv = (
                        po[:]
                        .rearrange("p (cs k) -> p cs k", cs=C * NS, k=NBASIS)
                        .unsqueeze(2)
                        .broadcast_to([128, C * NS, 2 * C, NBASIS])
                    )
                    lov = loBs.rearrange(
                        "p (cs d k) -> p cs d k", cs=C * NS, d=2 * C, k=NBASIS
                    )
                    nc.vector.tensor_tensor(zztv, pv, lov, op=ALU.mult)
                    zz1 = p_sm.tile([128, C * NS * 2 * C], F32, tag="zz1")
                    nc.vector.reduce_sum(
                        zz1[:],
                        zzt[:].rearrange(
                            "p (csd k) -> p csd k", csd=C * NS * 2 * C, k=NBASIS
                        ),
                        axis=mybir.AxisListType.X,
                    )
                    osl = ot[:, (b * NTT + tt) * W24 : (b * NTT + tt + 1) * W24]
                    nc.vector.reduce_sum(
                        osl.rearrange("p (sd) -> p sd", sd=W24),
                        zz1[:].rearrange("p (c sd) -> p sd c", c=C, sd=W24),
                        axis=mybir.AxisListType.X,
                    )
                    nc.vector.tensor_tensor(
                        osl, osl, lobb[:, tt * W24 : (tt + 1) * W24], op=ALU.add
                    )
                # softplus on this batch's std cols
                ng = NTT * NS
                sv = ot[:, b * NTT * W24 : (b + 1) * NTT * W24].rearrange(
                    "p (g d) -> p g d", g=ng, d=2 * C
                )[:, :, C:]
                av = p_sm.tile([128, ng * C], F32, tag="av")
                avv = av[:].rearrange("p (g d) -> p g d", g=ng, d=C)
                a1 = nc.scalar.activation(avv, sv, AF.Abs)
                add_dep_helper(a1.ins, ld, sync=False)
                rv = p_sm.tile([128, ng * C], F32, tag="rv")
                rvv = rv[:].rearrange("p (g d) -> p g d", g=ng, d=C)
                a2 = nc.scalar.activation(rvv, sv, AF.Relu)
                add_dep_helper(a2.ins, ld, sync=False)
                ew = p_sm.tile([128, ng * C], F32, tag="ew")
                a3 = nc.scalar.activation(ew[:], av[:], AF.Exp, scale=-1.0)
                add_dep_helper(a3.ins, ld, sync=False)
                lw_ = p_sm.tile([128, ng * C], F32, tag="lw_")
                a4 = nc.scalar.activation(lw_[:], ew[:], AF.Ln, bias=1.0)
                add_dep_helper(a4.ins, ld, sync=False)
                lvv = lw_[:].rearrange("p (g d) -> p g d", g=ng, d=C)
                nc.vector.tensor_tensor(sv, rvv, lvv, op=ALU.add)
                for tt in range(NTT):
                    nc.sync.dma_start(
                        d_out.ap()[:, b, tt * 128 : (tt + 1) * 128, :].rearrange(
                            "s p d -> p s d"
                        ),
                        ot[
                            :, (b * NTT + tt) * W24 : (b * NTT + tt + 1) * W24
                        ].rearrange("p (s d) -> p s d", s=NS, d=2 * C),
                    )

    nc.compile()
    return nc


def _prep(inputs):
    """Host-side sorting/pair-merging/packing. Returns (key, per-core in_maps)."""
    x = np.ascontiguousarray(inputs["x"], dtype=np.float32)
    y = np.ascontiguousarray(inputs["y"], dtype=np.float32)
    x_out = np.ascontiguousarray(inputs["x_out"], dtype=np.float32)
    x_grid = np.asarray(inputs["x_grid"], dtype=np.float32)
    eps_noise = np.asarray(inputs["eps_noise"], dtype=np.float32)
    enc_sigma = np.asarray(inputs["enc_sigma"], dtype=np.float64)
    int_sigma = np.asarray(inputs["int_sigma"], dtype=np.float64)
    gW = np.asarray(inputs["gW"], dtype=np.float32)
    gb = np.asarray(inputs["gb"], dtype=np.float32)
    w1 = np.asarray(inputs["w1"], dtype=np.float32)
    b1 = np.asarray(inputs["b1"], dtype=np.float32)
    w2 = np.asarray(inputs["w2"], dtype=np.float32)
    b2 = np.asarray(inputs["b2"], dtype=np.float32)
    w3 = np.asarray(inputs["w3"], dtype=np.float32)
    b3 = np.asarray(inputs["b3"], dtype=np.float32)
    linW = np.asarray(inputs["linW"], dtype=np.float32)
    linb = np.asarray(inputs["linb"], dtype=np.float32)
    loW = np.asarray(inputs["loW"], dtype=np.float32)
    lob = np.asarray(inputs["lob"], dtype=np.float32)

    nb, npts, _ = x.shape
    assert nb == NB and npts == NPTS
    m = x_grid.shape[1]
    njt = (m + 127) // 128
    g = x_grid[0, :, 0].astype(np.float64)
    g0 = float(g[0])
    gd = float((g[-1] - g[0]) / (m - 1))

    s_enc = np.exp(enc_sigma) + EPS
    alpha_enc = 1.0 / (np.sqrt(2.0) * s_enc)
    assert np.ptp(alpha_enc) < 1e-9 * abs(alpha_enc[0]), "enc_sigma must be uniform"
    s_int = np.exp(int_sigma) + EPS
    assert np.ptp(s_int) < 1e-12 * abs(s_int.flat[0]), "int_sigma must be uniform"
    alpha_int = float(1.0 / (np.sqrt(2.0) * s_int.flat[0]))
    _build.alpha_enc = [float(a) for a in alpha_enc]
    _build.alpha_int = alpha_int

    # ---- sort + pair-merge points per (b, c); global affine window lattice
    srt = np.sort(x.transpose(0, 2, 1), axis=2)
    idx = np.argsort(x.transpose(0, 2, 1), axis=2, kind="stable")
    ysrt = np.take_along_axis(y.transpose(0, 2, 1), idx, axis=2)
    xs_all = 0.5 * (srt[:, :, 0::2] + srt[:, :, 1::2])
    ys_all = ysrt[:, :, 0::2] + ysrt[:, :, 1::2]
    chunks = xs_all.reshape(NB, C, NCH, 128)
    reach = KREACH / alpha_enc.reshape(1, 3, 1)
    c_lo = np.ceil((chunks[:, :, :, 0] - reach - g0) / gd).astype(int)
    c_hi = np.floor((chunks[:, :, :, -1] + reach - g0) / gd).astype(int)
    ch_idx = np.arange(NCH)
    qfit = (c_lo[:, :, -1] + c_hi[:, :, -1] - c_lo[:, :, 0] - c_hi[:, :, 0]) / (
        2.0 * (NCH - 1)
    )
    q = int(round(float(np.median(qfit))))
    a = int((c_lo - q * ch_idx).min())
    whi = int((c_hi - q * ch_idx).max())
    wf = whi - a + 1
    off = min(a, 0)
    aoff = a - off
    mext = max(m, a + q * (NCH - 1) + wf) - off
    assert mext <= 512, f"psum extent {mext} > 512"
    assert wf <= 128, f"window {wf} too wide"

    # ---- packed device tensors ----
    o_bj = 0
    o_lbr = o_bj + njt
    o_low = o_lbr + 2 * NF
    o_lob = o_low + C * NS * 2 * C * NBASIS
    n128 = o_lob + NTT * W24
    o_gbn = 2 * RIN
    o_w1 = o_gbn + 1
    o_w2 = o_w1 + KW * ROUT
    o_w3 = o_w2 + KW * ROUT
    o_b123 = o_w3 + KW * ROUT
    o_linw = o_b123 + 3
    n32 = o_linw + 2 * NF
    o_xtr = 0
    o_eps = o_xtr + C * NTAR
    nb_ = o_eps + NF * NS
    o_gx = NBL * NCH * C

    p128 = np.zeros((128, n128), np.float32)
    gpad = np.zeros(njt * 128, np.float32)
    gpad[:m] = g.astype(np.float32)
    p128[:, o_bj : o_bj + njt] = (-alpha_int * gpad).reshape(njt, 128).T
    perm = np.array(
        [h * 15 + k * C + c for h in range(2) for c in range(C) for k in range(NBASIS)]
    )
    p128[:, o_lbr : o_lbr + 2 * NF] = linb[perm][None, :]
    lo = KAPPA * loW.reshape(NBASIS, C, 2 * C)  # (k, c, d)
    loBs_vec = (
        np.broadcast_to(lo.transpose(1, 2, 0)[:, None, :, :], (C, NS, 2 * C, NBASIS))
        .reshape(C * NS * 2 * C * NBASIS)
        .astype(np.float32)
    )
    p128[:, o_low : o_low + C * NS * 2 * C * NBASIS] = loBs_vec[None, :]
    p128[:, o_lob : o_lob + NTT * W24] = np.tile(lob, NTT * NS)[None, :]

    p32 = np.zeros((ROUT, n32), np.float32)
    p32[0:C, 0 : 2 * RIN] = np.concatenate([KAPPA * gW[0:3], gW[3:6]], axis=1)
    p32[0:RIN, o_gbn] = gb
    p32[0:RIN, o_w1 : o_w1 + KW * ROUT] = w1.transpose(1, 2, 0).reshape(RIN, -1)
    p32[0:ROUT, o_w2 : o_w2 + KW * ROUT] = w2.transpose(1, 2, 0).reshape(ROUT, -1)
    p32[0:ROUT, o_w3 : o_w3 + KW * ROUT] = w3.transpose(1, 2, 0).reshape(ROUT, -1)
    p32[0:ROUT, o_b123 : o_b123 + 3] = np.stack([b1, b2, b3], axis=1)
    p32[0:ROUT, o_linw : o_linw + 2 * NF] = linW[:, perm]

    ion = np.zeros((1, 2 + mext), np.float32)
    ion[0, 0] = EPS / KAPPA
    ion[0, 2:] = 1.0
    ion = ion.astype(ml_dtypes.bfloat16)

    xsr = xs_all.reshape(NB, C, NCH, 128).transpose(0, 3, 2, 1).reshape(NB, 128, -1)
    gxrow = (g0 + gd * (np.arange(mext) + off)).astype(np.float32)
    pball = np.empty((NB, 128, nb_), np.float32)
    pball[:, :, o_xtr : o_xtr + C * NTAR] = np.broadcast_to(
        x_out.transpose(0, 2, 1).reshape(NB, 1, C * NTAR), (NB, 128, C * NTAR)
    )
    pball[:, :, o_eps :] = np.broadcast_to(
        eps_noise.transpose(1, 2, 0).reshape(NB, 1, NF * NS), (NB, 128, NF * NS)
    )
    yi = np.empty((NB, 128, NCH * C * 2), np.float32)
    yi[:, :, 0::2] = 2.0
    yi[:, :, 1::2] = (
        ys_all.reshape(NB, C, NCH, 128).transpose(0, 3, 2, 1).reshape(NB, 128, -1)
    )
    yi = yi.astype(ml_dtypes.bfloat16)

    in_maps = []
    for core in range(NCORES):
        bsl = slice(core * NBL, (core + 1) * NBL)
        xgc = np.empty((128, NBL * NCH * C + mext), np.float32)
        for bl in range(NBL):
            xgc[:, bl * NCH * C : (bl + 1) * NCH * C] = xsr[core * NBL + bl]
        xgc[:, NBL * NCH * C :] = gxrow[None, :]
        in_maps.append(
            {
                "xg": xgc,
                "p128": p128,
                "p32": p32,
                "ion": ion,
                "pb": pball[bsl].copy(),
                "yi": np.ascontiguousarray(yi[bsl]),
            }
        )
    key = (m, q, aoff, wf, mext, -off, _build.alpha_int, tuple(_build.alpha_enc))
    return key, in_maps


def kernel(**inputs):
    key, in_maps = _prep(inputs)
    if key not in _CACHE:
        _CACHE[key] = _build(*key[:6])
    nc = _CACHE[key]
    res = bass_utils.run_bass_kernel_spmd(nc, in_maps, core_ids=list(range(NCORES)))
    outs = [res.results[c]["out"] for c in range(NCORES)]  # each (NS, NBL, NTAR, 6)
    full = np.concatenate(outs, axis=1)  # (NS, NB, NTAR, 6)
    return full.astype(np.float32)
